# revision 21
# baseline (speedup 1.0000x reference)
"""AddressAwareGNN (4-layer GAT + concat pooling + MLP) on 8 Trainium2 cores.

Sharding: nodes/edges partitioned by destination-node range (graph parallel).
Per layer: a fused projection emits node-major rows [256 feat | 8 a_src]
straight from the matmul (attention vectors folded into the weights on host,
so no separate attention matmul and no feature-major->node-major transposes),
AllGather of the 528B rows, then per-window edge aggregation: one SWDGE
indirect gather per 128-edge subtile (edges sorted by source row for HBM
locality), a_dst distributed via one-hot fp8 matmuls, exp on the scalar engine
(with a static -ln64 bias so fp16 h*exp products cannot overflow; the 1/64
cancels in the softmax), and one-hot fp8 matmuls for the segment-softmax
scatter-add. All 16-bit tensors are fp16 (not bf16) for the extra mantissa.
BatchNorm batch-stats via a small AllReduce, stats/apply chunked so they
overlap the aggregation tail and the next projection. Pooling: segment-sum via
one-hot matmul, per-graph max via dma_scatter_add into a zeroed graph-padded
buffer + transposing loads; classifier replicated on all cores.
"""
import os
import sys

sys.path.insert(0, "/opt/trn_rl_repo")

import heapq
import numpy as np
import ml_dtypes

import concourse.bass as bass
import concourse.mybir as mybir
import concourse.tile as tile
from concourse import bacc
from concourse.bass_utils import run_bass_kernel_spmd
from concourse.library_config import mlp as LIB_MLP
from concourse.masks import make_identity

BF16 = np.float16          # 16-bit activations: fp16 (more mantissa than bf16)
FP8 = ml_dtypes.float8_e4m3
F32 = mybir.dt.float32
BF = mybir.dt.float16
F8 = mybir.dt.float8e4
I32 = mybir.dt.int32
I16 = mybir.dt.int16
AF = mybir.ActivationFunctionType
ALU = mybir.AluOpType
AX = mybir.AxisListType

N, F_IN, H, HEADS, HD, L, G, NGF, NC = 50000, 64, 256, 8, 32, 4, 64, 32, 2
EPS = 1e-5
NCORES = 8
NLOC = N // NCORES          # 6250
NW = 49
WIN = 128
NLP = NW * WIN              # 6272 padded local rows
NGLOB = NLP * NCORES        # 50176
DROW = 264                  # table row: [256 feat | 8 a_src]
DHX = H + 2 * HEADS         # 272 meaningful columns
P = 128
PADG = 1024                 # max nodes per graph (gather padding)
NCH = (NLP + 511) // 512    # 13 feature-major column chunks


# ------------------------------------------------------------------ host prep
def _wrap16(idxs, cap):
    """Pack cap int16 indices into the [16, cap//16] column-major wrap."""
    arr = np.zeros((16, cap // 16), np.int16)
    j = np.arange(len(idxs))
    arr[j % 16, j // 16] = idxs
    return arr


def _prep(inputs):
    ei = np.asarray(inputs["edge_index"]).astype(np.int64)
    batch = np.asarray(inputs["batch"]).astype(np.int64)
    src = np.concatenate([ei[0], np.arange(N, dtype=np.int64)])
    dst = np.concatenate([ei[1], np.arange(N, dtype=np.int64)])
    order = np.argsort(dst, kind="stable")
    src, dst = src[order], dst[order]
    deg = np.bincount(dst, minlength=N)
    # edge-balanced core boundaries: cap per-core edges at NW*9*P so 9 subtiles
    # are reachable, while per-core node counts stay within the NLP padded rows
    pref = np.concatenate([[0], np.cumsum(deg)])
    EMAX = NW * 9 * P
    bounds = [0]
    for c in range(NCORES - 1):
        b = bounds[-1]
        e = int(np.searchsorted(pref, pref[b] + EMAX, side="right") - 1)
        bounds.append(min(e, b + NLP, N))
    bounds.append(N)
    if N - bounds[NCORES - 1] > NLP:
        bounds = list(np.arange(0, N + 1, NLOC))   # fallback: uniform split
    core_lo = np.searchsorted(dst, np.asarray(bounds))

    # per-core balanced assignment of dst nodes to (window, slot)
    raw_plans = []
    for c in range(NCORES):
        lo, hi = bounds[c], bounds[c + 1]
        cntc = hi - lo
        nodes = np.arange(lo, hi)
        d = deg[nodes]
        order_n = np.argsort(-d, kind="stable")
        base = cntc // NW
        cap = np.full(NW, base, np.int64)
        cap[:cntc - base * NW] += 1
        wload = np.zeros(NW, np.int64)
        win_nodes = [[] for _ in range(NW)]
        heap = [(0, w) for w in range(NW)]
        heapq.heapify(heap)
        for i in order_n:
            while True:
                load, w = heapq.heappop(heap)
                if len(win_nodes[w]) < cap[w]:
                    break
            win_nodes[w].append(i)
            wload[w] += d[i]
            if len(win_nodes[w]) < cap[w]:
                heapq.heappush(heap, (int(wload[w]), w))
        raw_plans.append((nodes, win_nodes, wload, d, cap))

    # repair pass: swap nodes between windows to cap every window's load one
    # subtile lower (windows are at node capacity, so only swaps rebalance)
    nat_spw = int(max(int(np.ceil(rp[2].max() / P)) for rp in raw_plans))
    target = (nat_spw - 1) * P
    for c in range(NCORES):
        nodes, win_nodes, wload, d, cap = raw_plans[c]
        if wload.sum() > target * NW:
            continue
        for _ in range(4 * NW):
            w = int(np.argmax(wload))
            if wload[w] <= target:
                break
            done = False
            for i in sorted(win_nodes[w], key=lambda n: -d[n]):
                need = wload[w] - target
                for w2 in np.argsort(wload):
                    if w2 == w:
                        continue
                    cands = [n2 for n2 in win_nodes[int(w2)] if d[i] - d[n2] >= 1
                             and wload[int(w2)] + d[i] - d[n2] <= target]
                    if not cands:
                        continue
                    j = min(cands, key=lambda n2: d[n2]) if need > 1 else \
                        max(cands, key=lambda n2: d[n2])
                    w2 = int(w2)
                    win_nodes[w].remove(i)
                    win_nodes[w2].remove(j)
                    win_nodes[w].append(j)
                    win_nodes[w2].append(i)
                    wload[w] += d[j] - d[i]
                    wload[w2] += d[i] - d[j]
                    done = True
                    break
                if done:
                    break
            if not done:
                break

    plans = []
    for c in range(NCORES):
        nodes, win_nodes, wload, d, cap = raw_plans[c]
        win_of = np.empty(len(nodes), np.int32)
        slot_of = np.empty(len(nodes), np.int32)
        for w in range(NW):
            for s, i in enumerate(win_nodes[w]):
                win_of[i] = w
                slot_of[i] = s
        plans.append((nodes, win_of, slot_of, wload))

    grow_of = np.full(N, -1, np.int64)
    for c, (nodes, win_of, slot_of, _) in enumerate(plans):
        grow_of[nodes] = c * NLP + win_of.astype(np.int64) * WIN + slot_of.astype(np.int64)

    SPW = int(max(int(np.ceil(p[3].max() / P)) for p in plans))
    NSUB = NW * SPW

    # per core: edges sorted by source table row within each window, p-major
    # flat layout (partition p holds a consecutive sorted run of SPW edges)
    per_core = []
    for c in range(NCORES):
        e0, e1 = core_lo[c], core_lo[c + 1]
        es, ed = src[e0:e1], dst[e0:e1]
        nodes, win_of, slot_of, _ = plans[c]
        lw = win_of[ed - bounds[c]]
        srow = grow_of[es]
        sl = slot_of[ed - bounds[c]]
        eorder = np.lexsort((srow, lw))
        lw, srow, sl = lw[eorder], srow[eorder], sl[eorder]
        wstart = np.searchsorted(lw, np.arange(NW + 1))
        SRCG = np.zeros((P, NSUB), np.int32)
        SST = np.zeros((P, NW, 2, SPW * P), FP8)
        ghost = np.ones((P, NW), np.float32)
        ghost[slot_of, win_of] = 0.0
        for w in range(NW):
            a, b = wstart[w], wstart[w + 1]
            k = b - a
            assert k <= SPW * P, f"window overflow {k}"
            j = np.arange(k)
            pp = j // SPW          # partition-major: p gets a sorted run
            kk = j % SPW
            SRCG[pp, w * SPW + kk] = srow[a:b]
            # S: [edge_p partition] x [sub*P + slot]
            SST[pp, w, 0, kk * P + sl[a:b]] = 1
            # ST: [slot partition] x [sub*P + edge_p]
            SST[sl[a:b], w, 1, kk * P + pp] = 1
        per_core.append(dict(SRCG=SRCG,
                             SST=np.ascontiguousarray(SST.reshape(P, NW, 2 * SPW * P)),
                             ghost=ghost))

    gs = np.searchsorted(batch, np.arange(G + 1))
    cnt = (gs[1:] - gs[:-1]).astype(np.float32)
    assert (gs[1:] - gs[:-1]).max() <= PADG
    glists = []
    for c in range(NCORES):
        lo, hi = bounds[c], bounds[c + 1]
        gl = [g for g in range(G) if gs[g] < hi and gs[g + 1] > lo]
        glists.append(gl)
    NG = max(len(gl) for gl in glists)

    for c in range(NCORES):
        nodes, win_of, slot_of, _ = plans[c]
        Sg = np.zeros((P, NW, G), BF16)
        Sg[slot_of, win_of, batch[nodes]] = 1
        per_core[c]["Sg"] = np.ascontiguousarray(Sg.reshape(P, NW * G))
        lo, hi = bounds[c], bounds[c + 1]
        # scatter targets: graph-slot-padded rows (slot s covers [s*PADG, (s+1)*PADG));
        # ghosts land uniquely in the junk tail at NG*PADG+
        gtgt = np.full((NG, 1), G, np.int32)
        slot_of_g = {}
        for i, g in enumerate(glists[c]):
            gtgt[i, 0] = g
            slot_of_g[g] = i
        lpos = np.empty(hi - lo, np.int64)
        for i in range(hi - lo):
            g = batch[lo + i]
            lpos[i] = slot_of_g[g] * PADG + (lo + i - max(gs[g], lo))
        nodepos = np.full((P, NW), -1, np.int64)
        nodepos[slot_of, win_of] = lpos
        gh_p, gh_w = np.where(nodepos < 0)
        nodepos[gh_p, gh_w] = NG * PADG + np.arange(len(gh_p))
        assert nodepos.max() < 32768
        # wrapped int16 for dma_scatter_add: flat j = w*128 + slot
        flat = nodepos.T.reshape(-1)          # [NW*P] with j = w*128+slot
        per_core[c]["npos"] = np.tile(_wrap16(flat, NW * P), (8, 1))
        per_core[c]["gtgt"] = gtgt

    def bf(x):
        return np.ascontiguousarray(np.asarray(x, np.float32)).astype(BF16)

    Wenc = bf(inputs["W_enc"])
    Wg = np.asarray(inputs["Wg"], np.float32)                  # [L, H, H]
    a_s = np.asarray(inputs["att_src"], np.float32)
    a_d = np.asarray(inputs["att_dst"], np.float32)
    Amat = np.zeros((L, H, 2 * HEADS), np.float32)
    for l in range(L):
        for h in range(HEADS):
            Amat[l, 32 * h:32 * h + 32, h] = a_s[l, h]
            Amat[l, 32 * h:32 * h + 32, HEADS + h] = a_d[l, h]
    WgA = np.einsum("lij,ljk->lik", Wg, Amat)                   # [L, H, 16]
    Wfull = np.concatenate([Wg, WgA], axis=2)                   # [L, H, 272]
    WgWa = np.ascontiguousarray(Wfull.reshape(L, 2, P, DHX)).astype(BF16)

    bnp = np.zeros((L + 1, P, 2, 2), np.float32)
    pairs = [(inputs["g_enc"], inputs["be_enc"])] + [(inputs["bn_g"][l], inputs["bn_b"][l]) for l in range(L)]
    for i, (g_, b_) in enumerate(pairs):
        g_, b_ = np.asarray(g_, np.float32), np.asarray(b_, np.float32)
        bnp[i, :, 0, 0], bnp[i, :, 1, 0] = g_[:P], g_[P:]
        bnp[i, :, 0, 1], bnp[i, :, 1, 1] = b_[:P], b_[P:]
    W1 = np.asarray(inputs["W1"], np.float32)
    W1p = np.zeros((7, P, 2 * H), np.float32)
    for kt in range(7):
        r = W1[kt * P:(kt + 1) * P]
        W1p[kt, :r.shape[0]] = r
    W2p = np.ascontiguousarray(np.asarray(inputs["W2"], np.float32)).reshape(4, P, H)
    W3p = np.ascontiguousarray(np.asarray(inputs["W3"], np.float32)).reshape(2, P, NC)
    bn1p = np.zeros((P, 4, 2), np.float32)
    bn1p[:, :, 0] = np.asarray(inputs["g1"], np.float32).reshape(4, P).T
    bn1p[:, :, 1] = np.asarray(inputs["be1"], np.float32).reshape(4, P).T
    bn2p = np.zeros((P, 2, 2), np.float32)
    bn2p[:, :, 0] = np.asarray(inputs["g2"], np.float32).reshape(2, P).T
    bn2p[:, :, 1] = np.asarray(inputs["be2"], np.float32).reshape(2, P).T
    b3 = np.asarray(inputs["b3"], np.float32).reshape(NC, 1)
    gf = np.ascontiguousarray(np.asarray(inputs["graph_features"], np.float32).reshape(G, NGF))
    cntr = (1.0 / cnt).reshape(G, 1).astype(np.float32)

    x = np.asarray(inputs["x"], np.float32)
    in_maps = []
    for c in range(NCORES):
        nodes, win_of, slot_of, _ = plans[c]
        lid = win_of.astype(np.int64) * WIN + slot_of.astype(np.int64)
        xT = np.zeros((F_IN, NLP), np.float32)
        xT[:, lid] = x[nodes].T
        m = dict(per_core[c])
        m.update(xT=xT.astype(BF16), Wenc=Wenc, WgWa=WgWa, bnp=bnp,
                 W1p=W1p, W2p=W2p, W3p=W3p, bn1p=bn1p, bn2p=bn2p, b3=b3,
                 gf=gf, cntr=cntr)
        in_maps.append(m)
    cfg = dict(SPW=SPW, NSUB=NSUB, NG=NG)
    return in_maps, cfg


# ------------------------------------------------------------------ builder
def _build(nc, cfg, debug=False):
    RG = [list(range(NCORES))]
    SPW, NSUB, NG = cfg["SPW"], cfg["NSUB"], cfg["NG"]

    if debug:
        d_dbg_hx = nc.dram_tensor("dbg_hx", [NLP, DROW], BF, kind="ExternalOutput")
        d_dbg_ad = nc.dram_tensor("dbg_ad", [P, NW * HEADS], BF, kind="ExternalOutput")
        d_dbg_zT = nc.dram_tensor("dbg_zT", [P, 2 * NLP], F32, kind="ExternalOutput")
        d_dbg_G = nc.dram_tensor("dbg_G", [P, SPW * DROW], BF, kind="ExternalOutput")

    d_SRCG = nc.dram_tensor("SRCG", [P, NSUB], I32, kind="ExternalInput")
    d_SST = nc.dram_tensor("SST", [P, NW, 2 * SPW * P], F8, kind="ExternalInput")
    d_gh = nc.dram_tensor("ghost", [P, NW], F32, kind="ExternalInput")
    d_np = nc.dram_tensor("npos", [P, NW * P // 16], I16, kind="ExternalInput")
    d_Sg = nc.dram_tensor("Sg", [P, NW * G], BF, kind="ExternalInput")
    d_gtgt = nc.dram_tensor("gtgt", [NG, 1], I32, kind="ExternalInput")
    d_cnt = nc.dram_tensor("cntr", [G, 1], F32, kind="ExternalInput")
    d_xT = nc.dram_tensor("xT", [F_IN, NLP], BF, kind="ExternalInput")
    d_Wenc = nc.dram_tensor("Wenc", [F_IN, H], BF, kind="ExternalInput")
    d_WgWa = nc.dram_tensor("WgWa", [L, 2, P, DHX], BF, kind="ExternalInput")
    d_bnp = nc.dram_tensor("bnp", [L + 1, P, 2, 2], F32, kind="ExternalInput")
    d_W1 = nc.dram_tensor("W1p", [7, P, 2 * H], F32, kind="ExternalInput")
    d_W2 = nc.dram_tensor("W2p", [4, P, H], F32, kind="ExternalInput")
    d_W3 = nc.dram_tensor("W3p", [2, P, NC], F32, kind="ExternalInput")
    d_bn1 = nc.dram_tensor("bn1p", [P, 4, 2], F32, kind="ExternalInput")
    d_bn2 = nc.dram_tensor("bn2p", [P, 2, 2], F32, kind="ExternalInput")
    d_b3 = nc.dram_tensor("b3", [NC, 1], F32, kind="ExternalInput")
    d_gf = nc.dram_tensor("gf", [G, NGF], F32, kind="ExternalInput")
    d_out = nc.dram_tensor("out", [G, NC], F32, kind="ExternalOutput")

    with tile.TileContext(nc, trace_sim=False) as tc:
        with (
            tc.tile_pool(name="sb", bufs=1) as sb,
            tc.tile_pool(name="dr", bufs=2, space="DRAM") as dr,
        ):
            nc.gpsimd.load_library(LIB_MLP)
            idf = sb.tile([P, P], F32)
            make_identity(nc, idf[:])
            idb = sb.tile([P, P], BF)
            make_identity(nc, idb[:])
            eps_sb = sb.tile([P, 1], F32)
            nc.vector.memset(eps_sb[:], EPS)
            # static softmax downscale: exp(e - ln 64). The 1/64 cancels in
            # numerator/denominator; keeps fp16 h*exp products under 65504.
            nl64_sb = sb.tile([P, 1], F32)
            nc.vector.memset(nl64_sb[:], -4.15888308)

            srcg_sb = sb.tile([P, NSUB], I32)
            nc.sync.dma_start(srcg_sb[:], d_SRCG[:])
            ghost_sb = sb.tile([P, NW], F32)
            nc.sync.dma_start(ghost_sb[:], d_gh[:])
            bnp_sb = sb.tile([P, L + 1, 2, 2], F32)
            nc.sync.dma_start(bnp_sb[:], d_bnp[:].rearrange("l p b k -> p l b k"))

            hA = sb.tile([P, 2, NLP], BF)
            hB = sb.tile([P, 2, NLP], BF)
            d_prev = dr.tile([P, 2, NLP], BF, tag="prev", bufs=1)

            # pooling preloads (no deps; issued early so the pooling phase
            # doesn't pay for them)
            sg_sb = sb.tile([P, NW * G], BF)
            nc.sync.dma_start(sg_sb[:], d_Sg[:])
            np_sb = sb.tile([P, NW * P // 16], I16)
            nc.sync.dma_start(np_sb[:], d_np[:])
            NRPL = NG * PADG + NLP
            hf_loc = dr.tile([NRPL, H], BF, tag="hfloc", bufs=1)
            zt0 = sb.tile([P, 2048], BF)
            nc.vector.memset(zt0[:], 0.0)
            for r0 in range(0, NRPL - 1024 + 1, 1024):
                nc.sync.dma_start(
                    hf_loc[r0:r0 + 1024, :].rearrange("(a b) h -> a (b h)", a=P),
                    zt0[:])
            if (NRPL // 1024) * 1024 < NRPL:
                nc.sync.dma_start(
                    hf_loc[NRPL - 1024:NRPL, :].rearrange("(a b) h -> a (b h)", a=P),
                    zt0[:])

            with tc.tile_pool(name="zp", bufs=1) as zp:
                zT = zp.tile([P, 2, NLP], F32)

                def batchnorm_relu(lay, dst_tile, scratch):
                    NCK = 4
                    CK = NLP // NCK
                    stats4 = zp.tile([P, NCK, 4], F32, tag="bnstats4", bufs=2)
                    for ck in range(NCK):
                        c0, c1 = ck * CK, (ck + 1) * CK
                        nc.vector.reduce_sum(stats4[:, ck, 0:1], zT[:, 0, c0:c1], axis=AX.X)
                        nc.vector.reduce_sum(stats4[:, ck, 1:2], zT[:, 1, c0:c1], axis=AX.X)
                        nc.scalar.activation(scratch[:, 0, c0:c1], zT[:, 0, c0:c1], AF.Square,
                                             accum_out=stats4[:, ck, 2:3])
                        nc.scalar.activation(scratch[:, 1, c0:c1], zT[:, 1, c0:c1], AF.Square,
                                             accum_out=stats4[:, ck, 3:4])
                    stats = zp.tile([P, 4], F32, tag="bnstats", bufs=2)
                    nc.vector.tensor_add(stats[:], stats4[:, 0, :], stats4[:, 1, :])
                    nc.vector.tensor_add(stats4[:, 2, :], stats4[:, 2, :], stats4[:, 3, :])
                    nc.vector.tensor_add(stats[:], stats[:], stats4[:, 2, :])
                    sin = dr.tile([P, 4], F32, tag="bnin")
                    sout = dr.tile([P, 4], F32, tag="bnout", addr_space="Shared")
                    nc.sync.dma_start(sin[:], stats[:])
                    nc.gpsimd.collective_compute("AllReduce", ALU.add, replica_groups=RG,
                                                 ins=[sin[:].opt()], outs=[sout[:].opt()])
                    st = zp.tile([P, 4], F32, tag="bnst", bufs=2)
                    nc.sync.dma_start(st[:], sout[:])
                    mu = zp.tile([P, 2], F32, tag="bnmu", bufs=2)
                    nc.vector.tensor_scalar_mul(mu[:], st[:, 0:2], 1.0 / N)
                    var = zp.tile([P, 2], F32, tag="bnvar", bufs=2)
                    nc.vector.tensor_scalar_mul(var[:], st[:, 2:4], 1.0 / N)
                    musq = zp.tile([P, 2], F32, tag="bnmusq", bufs=2)
                    nc.vector.tensor_mul(musq[:], mu[:], mu[:])
                    nc.vector.tensor_sub(var[:], var[:], musq[:])
                    rs = zp.tile([P, 2], F32, tag="bnrs", bufs=2)
                    nc.scalar.activation(rs[:], var[:], AF.Sqrt, bias=eps_sb[:, 0:1])
                    nc.vector.reciprocal(rs[:], rs[:])
                    Sc = zp.tile([P, 2], F32, tag="bnS", bufs=2)
                    nc.vector.tensor_mul(Sc[:], rs[:], bnp_sb[:, lay, :, 0])
                    Bi = zp.tile([P, 2], F32, tag="bnB", bufs=2)
                    nc.vector.tensor_mul(Bi[:], mu[:], Sc[:])
                    nc.vector.tensor_sub(Bi[:], bnp_sb[:, lay, :, 1], Bi[:])
                    for cc in range(4):
                        c0, c1 = cc * (NLP // 4), (cc + 1) * (NLP // 4)
                        for b in range(2):
                            nc.scalar.activation(dst_tile[:, b, c0:c1], zT[:, b, c0:c1], AF.Relu,
                                                 bias=Bi[:, b:b + 1], scale=Sc[:, b:b + 1])

                # ---------------- encoder ----------------
                with (
                    tc.tile_pool(name="encp", bufs=1) as ep,
                    tc.tile_pool(name="psenc", bufs=2, space="PSUM") as ps_enc,
                ):
                    xT_sb = ep.tile([F_IN, NLP], BF)
                    nc.sync.dma_start(xT_sb[:], d_xT[:])
                    wenc_sb = ep.tile([F_IN, H], BF)
                    nc.sync.dma_start(wenc_sb[:], d_Wenc[:])
                    for ch in range(NCH):
                        f0 = ch * 512
                        F = min(512, NLP - f0)
                        for kb in range(2):
                            pz = ps_enc.tile([P, 512], F32, tag="mm")
                            nc.tensor.matmul(pz[:, :F], wenc_sb[:, kb * P:(kb + 1) * P],
                                             xT_sb[:, f0:f0 + F], start=True, stop=True)
                            nc.scalar.activation(zT[:, kb, f0:f0 + F], pz[:, :F], AF.Copy)
                    batchnorm_relu(0, hA, hB)

                # ---------------- GAT layers ----------------
                with (
                    tc.tile_pool(name="edge", bufs=1) as eb,
                    tc.tile_pool(name="pspj", bufs=2, space="PSUM") as ps_pj,
                    tc.tile_pool(name="pswin", bufs=2, space="PSUM") as ps_win,
                    tc.tile_pool(name="pstr", bufs=2, space="PSUM") as ps_tr,
                ):
                    adst = eb.tile([P, NW, HEADS], BF, tag="adst", bufs=2)
                    for l in range(L):
                        hin = hA if l % 2 == 0 else hB
                        hout = hB if l % 2 == 0 else hA
                        wg_sb = eb.tile([P, 2, DHX], BF, tag="wg", bufs=2)
                        nc.sync.dma_start(wg_sb[:], d_WgWa[l].rearrange("t p k -> p t k"))

                        # fused projection: node-major [128, 272] per window
                        hx_loc = dr.tile([NLP, DROW], BF, tag="hxloc")
                        for w in range(NW):
                            n0 = w * WIN
                            pz = ps_pj.tile([P, DHX], F32, tag="pj")
                            for jt in range(2):
                                nc.tensor.matmul(pz[:], hin[:, jt, n0:n0 + P],
                                                 wg_sb[:, jt, :], start=(jt == 0), stop=(jt == 1))
                            hxw = eb.tile([P, DHX], BF, tag="hxw", bufs=3)
                            nc.scalar.activation(hxw[:], pz[:], AF.Copy)
                            nc.vector.tensor_copy(adst[:, w, :], hxw[:, H + HEADS:DHX])
                            nc.sync.dma_start(hx_loc[n0:n0 + P, :], hxw[:, 0:DROW])

                        hx_full = dr.tile([NCORES, NLP, DROW], BF, tag="hxfull", addr_space="Shared")
                        nc.gpsimd.collective_compute("AllGather", ALU.bypass, replica_groups=RG,
                                                     ins=[hx_loc[:].opt()],
                                                     outs=[hx_full[:].opt()])
                        tab = hx_full[:].rearrange("c n d -> (c n) d")

                        for w in range(NW):
                            sst = eb.tile([P, 2, SPW * P], F8, tag="sst", bufs=6)
                            nc.sync.dma_start(sst[:].rearrange("p a b -> p (a b)"), d_SST[:, w])
                            Gt = eb.tile([P, SPW, DROW], BF, tag="G", bufs=6)
                            for k in range(SPW):
                                nc.gpsimd.indirect_dma_start(
                                    out=Gt[:, k, :], out_offset=None,
                                    in_=tab,
                                    in_offset=bass.IndirectOffsetOnAxis(
                                        ap=srcg_sb[:, w * SPW + k:w * SPW + k + 1], axis=0))
                            if debug and l == 0 and w == 0:
                                nc.sync.dma_start(d_dbg_G[:], Gt[:].rearrange("p j d -> p (j d)"))
                            T = ps_win.tile([P, 264 + SPW * HEADS], F32, tag="win")
                            for k in range(SPW):
                                nc.tensor.matmul(T[:, 264 + k * HEADS:264 + (k + 1) * HEADS],
                                                 sst[:, 1, k * P:(k + 1) * P],
                                                 adst[:, w, :], start=True, stop=True)
                            asc = eb.tile([P, SPW, HEADS], F32, tag="asc", bufs=3)
                            nc.vector.tensor_copy(asc[:], Gt[:, :, H:H + HEADS])
                            et = eb.tile([P, SPW, HEADS], F32, tag="et", bufs=3)
                            nc.vector.tensor_add(
                                et[:], asc[:],
                                T[:, 264:264 + SPW * HEADS].rearrange("p (j h) -> p j h", h=HEADS))
                            et3 = eb.tile([P, SPW, HEADS], F32, tag="et3", bufs=3)
                            nc.vector.tensor_scalar_mul(et3[:], et[:], 0.2)
                            nc.vector.tensor_max(et3[:], et3[:], et[:])
                            etb = eb.tile([P, SPW, HEADS], BF, tag="etb", bufs=2)
                            nc.scalar.activation(etb[:], et3[:], AF.Exp, bias=nl64_sb[:, 0:1])
                            # expanded exp on the scalar engine keeps the big
                            # DVE multiply fully contiguous
                            exb = eb.tile([P, SPW, HEADS, HD], BF, tag="exb", bufs=2)
                            nc.scalar.activation(
                                exb[:], et3[:, :, :, None].to_broadcast([P, SPW, HEADS, HD]),
                                AF.Exp, bias=nl64_sb[:, 0:1])
                            # exp into the a_src slot: feeds the denominator
                            # columns of the scatter-add matmul
                            nc.vector.tensor_copy(Gt[:, :, H:H + HEADS], etb[:])
                            gv = Gt[:, :, 0:H].rearrange("p j (h d) -> p j h d", h=HEADS)
                            nc.vector.tensor_mul(gv[:], gv[:], exb[:])
                            for k in range(SPW):
                                nc.tensor.matmul(T[:, 0:264], sst[:, 0, k * P:(k + 1) * P],
                                                 Gt[:, k, 0:264],
                                                 start=(k == 0), stop=(k == SPW - 1))
                            den = eb.tile([P, HEADS], F32, tag="den", bufs=3)
                            nc.vector.tensor_scalar(den[:], T[:, 256:264],
                                                    scalar1=ghost_sb[:, w:w + 1],
                                                    scalar2=None, op0=ALU.add)
                            nc.vector.reciprocal(den[:], den[:])
                            rcx = eb.tile([P, HEADS, HD], F32, tag="rcx", bufs=3)
                            nc.scalar.activation(
                                rcx[:], den[:, :, None].to_broadcast([P, HEADS, HD]), AF.Copy)
                            zw = eb.tile([P, H], F32, tag="zw", bufs=3)
                            nc.vector.tensor_mul(
                                zw[:].rearrange("p (h d) -> p h d", h=HEADS),
                                T[:, 0:H].rearrange("p (h d) -> p h d", h=HEADS),
                                rcx[:])
                            for b in range(2):
                                pt = ps_tr.tile([P, P], F32, tag="trf")
                                nc.tensor.transpose(out=pt[:], in_=zw[:, b * P:(b + 1) * P],
                                                    identity=idf[:])
                                if b == 0:
                                    nc.scalar.activation(zT[:, b, w * WIN:w * WIN + P], pt[:], AF.Copy)
                                else:
                                    nc.vector.tensor_copy(zT[:, b, w * WIN:w * WIN + P], pt[:])

                        if debug and l == 0:
                            nc.sync.dma_start(d_dbg_hx[:], hx_loc[:])
                            nc.sync.dma_start(d_dbg_ad[:], adst[:].rearrange("p w h -> p (w h)"))
                            nc.sync.dma_start(d_dbg_zT[:], zT[:].rearrange("p b n -> p (b n)"))
                        batchnorm_relu(l + 1, hout, hin)
                        if l == 1:
                            nc.sync.dma_start(d_prev[:], hout[:])
                        if l == 2:
                            for b in range(2):
                                for cc in range(7):
                                    c0 = cc * 896
                                    cw = min(896, NLP - c0)
                                    psc = eb.tile([P, 896], BF, tag="prevc", bufs=4)
                                    nc.sync.dma_start(psc[:, :cw], d_prev[:, b, c0:c0 + cw])
                                    nc.vector.tensor_add(hout[:, b, c0:c0 + cw], hout[:, b, c0:c0 + cw],
                                                         psc[:, :cw])

            # ---------------- pooling ----------------
            hfin = hA if L % 2 == 0 else hB
            with (
                tc.tile_pool(name="poolp", bufs=1) as pb,
                tc.tile_pool(name="psmm", bufs=2, space="PSUM") as ps_mm,
                tc.tile_pool(name="pstr2", bufs=2, space="PSUM") as ps_tr,
            ):
                hstage = pb.tile([P, NW, H], BF)
                pp0 = ps_mm.tile([G, H], F32, tag="mm")
                for w in range(NW):
                    n0 = w * WIN
                    for b in range(2):
                        pt = ps_tr.tile([P, P], BF, tag="trb", bufs=2)
                        nc.tensor.transpose(out=pt[:], in_=hfin[:, b, n0:n0 + P], identity=idb[:])
                        nc.vector.tensor_copy(hstage[:, w, b * P:(b + 1) * P], pt[:])
                    nc.tensor.matmul(pp0[:], sg_sb[:, w * G:(w + 1) * G], hstage[:, w, :],
                                     start=(w == 0), stop=(w == NW - 1))

                # per-graph max: scatter-add node rows into the zeroed
                # graph-slot-padded buffer (h >= 0 post-ReLU, so 0-pads are
                # neutral for max), then dma-transpose each slot and reduce
                HWN = 24 * P
                nc.gpsimd.dma_scatter_add(
                    hf_loc[:], hstage[:, 0:24, :], np_sb[:, 0:HWN // 16],
                    HWN, HWN, H)
                nc.gpsimd.dma_scatter_add(
                    hf_loc[:], hstage[:, 24:NW, :], np_sb[:, HWN // 16:NW * P // 16],
                    (NW - 24) * P, (NW - 24) * P, H)

                pmax = pb.tile([P, 2, NG], F32)
                for i in range(NG):
                    gt = pb.tile([P, 2, PADG], BF, tag="gt", bufs=2)
                    for b in range(2):
                        nc.sync.dma_start_transpose(
                            gt[:, b, :], hf_loc[i * PADG:(i + 1) * PADG, b * P:(b + 1) * P])
                    for b in range(2):
                        nc.vector.reduce_max(pmax[:, b, i:i + 1], gt[:, b, :], axis=AX.X)

                # rows: [G, 2H] = [sum | max]; combine across cores with one AG
                pmax_rows = pb.tile([P, H], F32)
                for b in range(2):
                    pt = ps_tr.tile([P, P], F32, tag="trf")
                    nc.tensor.transpose(out=pt[0:NG, 0:P], in_=pmax[:, b, :], identity=idf[:])
                    nc.scalar.activation(pmax_rows[0:NG, b * P:(b + 1) * P], pt[0:NG, 0:P], AF.Copy)

                pin_sb = pb.tile([G, 2 * H], F32)
                nc.scalar.activation(pin_sb[:, 0:H], pp0[:], AF.Copy)
                nc.vector.memset(pin_sb[:, H:2 * H], 0.0)
                pin = dr.tile([G + 1, 2 * H], F32, tag="pin")
                nc.sync.dma_start(pin[0:G, :], pin_sb[:])
                gtgt_sb = pb.tile([NG, 1], I32)
                nc.sync.dma_start(gtgt_sb[:], d_gtgt[:])
                nc.gpsimd.indirect_dma_start(
                    out=pin[:], out_offset=bass.IndirectOffsetOnAxis(ap=gtgt_sb[:], axis=0),
                    in_=pmax_rows[0:NG, :], in_offset=None, element_offset=H)

                pfull_d = dr.tile([NCORES, G, 2 * H], F32, tag="pfull", addr_space="Shared")
                nc.gpsimd.collective_compute("AllGather", ALU.bypass, replica_groups=RG,
                                             ins=[pin[0:G, :].opt()], outs=[pfull_d[:].opt()])
                pf8 = pb.tile([G, NCORES, 2 * H], F32)
                nc.sync.dma_start(pf8[:], pfull_d[:].rearrange("c g h -> g c h"))
                for cc in range(1, NCORES):
                    nc.vector.tensor_add(pf8[:, 0, 0:H], pf8[:, 0, 0:H], pf8[:, cc, 0:H])
                    nc.vector.tensor_max(pf8[:, 0, H:2 * H], pf8[:, 0, H:2 * H], pf8[:, cc, H:2 * H])

                # pooled [G, 800] = [mean | max | sum | gf]
                pooled = pb.tile([G, 3 * H + NGF], F32)
                cnt_sb = pb.tile([G, 1], F32)
                nc.sync.dma_start(cnt_sb[:], d_cnt[:])
                nc.vector.tensor_scalar(pooled[:, 0:H], pf8[:, 0, 0:H], scalar1=cnt_sb[:],
                                        scalar2=None, op0=ALU.mult)
                nc.vector.tensor_copy(pooled[:, H:2 * H], pf8[:, 0, H:2 * H])
                nc.vector.tensor_copy(pooled[:, 2 * H:3 * H], pf8[:, 0, 0:H])
                nc.sync.dma_start(pooled[:, 3 * H:], d_gf[:])

                # ---------------- classifier (replicated) ----------------
                pT = pb.tile([P, 7, G], F32)
                nc.vector.memset(pT[:], 0)
                for t in range(7):
                    w_ = min(P, 3 * H + NGF - t * P)
                    pt = ps_mm.tile([P, P], F32, tag="mm")
                    nc.tensor.transpose(out=pt[0:w_, 0:G], in_=pooled[:, t * P:t * P + w_],
                                        identity=idf[0:G, 0:G])
                    nc.scalar.activation(pT[:w_, t, :], pt[:w_, 0:G], AF.Copy)

                w1_sb = pb.tile([P, 7, 2 * H], F32)
                nc.sync.dma_start(w1_sb[:], d_W1[:].rearrange("t p k -> p t k"))
                bn1_sb = pb.tile([P, 4, 2], F32)
                nc.sync.dma_start(bn1_sb[:], d_bn1[:])
                z1 = pb.tile([P, 4, G], F32)

                def mlp_bn(zt, nblk, bnsb, ngraph=G):
                    for b in range(nblk):
                        s_ = pb.tile([P, 1], F32, tag="cbs", bufs=2)
                        nc.vector.reduce_sum(s_[:], zt[:, b, :], axis=AX.X)
                        sqt = pb.tile([P, G], F32, tag="cbsq", bufs=2)
                        q_ = pb.tile([P, 1], F32, tag="cbq", bufs=2)
                        nc.scalar.activation(sqt[:], zt[:, b, :], AF.Square, accum_out=q_[:])
                        mu = pb.tile([P, 1], F32, tag="cbmu", bufs=2)
                        nc.vector.tensor_scalar_mul(mu[:], s_[:], 1.0 / ngraph)
                        var = pb.tile([P, 1], F32, tag="cbvar", bufs=2)
                        nc.vector.tensor_scalar_mul(var[:], q_[:], 1.0 / ngraph)
                        ms = pb.tile([P, 1], F32, tag="cbms", bufs=2)
                        nc.vector.tensor_mul(ms[:], mu[:], mu[:])
                        nc.vector.tensor_sub(var[:], var[:], ms[:])
                        rs = pb.tile([P, 1], F32, tag="cbrs", bufs=2)
                        nc.scalar.activation(rs[:], var[:], AF.Sqrt, bias=eps_sb[:, 0:1])
                        nc.vector.reciprocal(rs[:], rs[:])
                        Sc = pb.tile([P, 1], F32, tag="cbS", bufs=2)
                        nc.vector.tensor_mul(Sc[:], rs[:], bnsb[:, b, 0:1])
                        Bi = pb.tile([P, 1], F32, tag="cbB", bufs=2)
                        nc.vector.tensor_mul(Bi[:], mu[:], Sc[:])
                        nc.vector.tensor_sub(Bi[:], bnsb[:, b, 1:2], Bi[:])
                        nc.scalar.activation(zt[:, b, :], zt[:, b, :], AF.Relu,
                                             bias=Bi[:], scale=Sc[:])

                for mb in range(4):
                    pz = ps_mm.tile([P, 512], F32, tag="mm")
                    for kt in range(7):
                        nc.tensor.matmul(pz[:, 0:G], w1_sb[:, kt, mb * P:(mb + 1) * P],
                                         pT[:, kt, :], start=(kt == 0), stop=(kt == 6))
                    nc.scalar.activation(z1[:, mb, :], pz[:, 0:G], AF.Copy)
                mlp_bn(z1, 4, bn1_sb)

                w2_sb = pb.tile([P, 4, H], F32)
                nc.sync.dma_start(w2_sb[:], d_W2[:].rearrange("t p k -> p t k"))
                bn2_sb = pb.tile([P, 2, 2], F32)
                nc.sync.dma_start(bn2_sb[:], d_bn2[:])
                z2 = pb.tile([P, 2, G], F32)
                for mb in range(2):
                    pz = ps_mm.tile([P, 512], F32, tag="mm")
                    for kt in range(4):
                        nc.tensor.matmul(pz[:, 0:G], w2_sb[:, kt, mb * P:(mb + 1) * P],
                                         z1[:, kt, :], start=(kt == 0), stop=(kt == 3))
                    nc.scalar.activation(z2[:, mb, :], pz[:, 0:G], AF.Copy)
                mlp_bn(z2, 2, bn2_sb)

                w3_sb = pb.tile([P, 2, NC], F32)
                nc.sync.dma_start(w3_sb[:], d_W3[:].rearrange("t p k -> p t k"))
                b3_sb = pb.tile([NC, 1], F32)
                nc.sync.dma_start(b3_sb[:], d_b3[:])
                pz3 = ps_mm.tile([P, 512], F32, tag="mm")
                for kt in range(2):
                    nc.tensor.matmul(pz3[0:NC, 0:G], w3_sb[:, kt, :], z2[:, kt, :],
                                     start=(kt == 0), stop=(kt == 1))
                z3 = pb.tile([NC, G], F32)
                nc.scalar.activation(z3[:], pz3[0:NC, 0:G], AF.Identity, bias=b3_sb[:, 0:1])
                nc.sync.dma_start(d_out[:].rearrange("g c -> c g"), z3[:])
    return nc


_CACHE = {}


def _get_compiled(cfg, debug=False):
    key = (cfg["SPW"], cfg["NSUB"], cfg["NG"], debug)
    if key not in _CACHE:
        nc = bacc.Bacc("TRN2", target_bir_lowering=False, debug=False,
                       num_devices=NCORES, dynamic_dma_scratch_size=24576)
        _build(nc, cfg, debug=debug)
        nc.compile()
        _CACHE[key] = nc
    return _CACHE[key]


def kernel(debug=False, _want_results=False, **inputs):
    in_maps, cfg = _prep(inputs)
    nc = _get_compiled(cfg)
    res = run_bass_kernel_spmd(nc, in_maps, core_ids=list(range(NCORES)))
    out = np.asarray(res.results[0]["out"], np.float32)
    if _want_results:
        return out, res
    return out


# revision 25
# speedup vs baseline: 1.0897x; 1.0897x over previous
"""AddressAwareGNN (4-layer GAT + concat pooling + MLP) on 8 Trainium2 cores.

Sharding: nodes/edges partitioned by destination-node range (graph parallel).
Per layer: a fused projection emits node-major rows [256 feat | 8 a_src]
straight from the matmul (attention vectors folded into the weights on host,
so no separate attention matmul and no feature-major->node-major transposes),
AllGather of the 528B rows, then per-window edge aggregation: one SWDGE
indirect gather per 128-edge subtile (edges sorted by source row for HBM
locality), a_dst distributed via one-hot fp8 matmuls, exp on the scalar engine
(with a static -ln64 bias so fp16 h*exp products cannot overflow; the 1/64
cancels in the softmax), and one-hot fp8 matmuls for the segment-softmax
scatter-add. All 16-bit tensors are fp16 (not bf16) for the extra mantissa.
BatchNorm batch-stats via a small AllReduce, stats/apply chunked so they
overlap the aggregation tail and the next projection. Pooling: segment-sum via
one-hot matmul, per-graph max via dma_scatter_add into a zeroed graph-padded
buffer + transposing loads; classifier replicated on all cores.
"""
import os
import sys

sys.path.insert(0, "/opt/trn_rl_repo")

import heapq
import numpy as np
import ml_dtypes

import concourse.bass as bass
import concourse.mybir as mybir
import concourse.tile as tile
from concourse import bacc
from concourse.bass_utils import run_bass_kernel_spmd
from concourse.library_config import mlp as LIB_MLP
from concourse.masks import make_identity

BF16 = np.float16          # 16-bit activations: fp16 (more mantissa than bf16)
FP8 = ml_dtypes.float8_e4m3
F32 = mybir.dt.float32
BF = mybir.dt.float16
F8 = mybir.dt.float8e4
I32 = mybir.dt.int32
I16 = mybir.dt.int16
AF = mybir.ActivationFunctionType
ALU = mybir.AluOpType
AX = mybir.AxisListType

N, F_IN, H, HEADS, HD, L, G, NGF, NC = 50000, 64, 256, 8, 32, 4, 64, 32, 2
EPS = 1e-5
NCORES = 8
NLOC = N // NCORES          # 6250
NW = 49
WIN = 128
NLP = NW * WIN              # 6272 padded local rows
NGLOB = NLP * NCORES        # 50176
DROW = 264                  # table row: [256 feat | 8 a_src]
DHX = H + 2 * HEADS         # 272 meaningful columns
P = 128
PADG = 1024                 # max nodes per graph (gather padding)
NCH = (NLP + 511) // 512    # 13 feature-major column chunks


# ------------------------------------------------------------------ host prep
def _wrap16(idxs, cap):
    """Pack cap int16 indices into the [16, cap//16] column-major wrap."""
    arr = np.zeros((16, cap // 16), np.int16)
    j = np.arange(len(idxs))
    arr[j % 16, j // 16] = idxs
    return arr


def _prep(inputs):
    ei = np.asarray(inputs["edge_index"]).astype(np.int64)
    batch = np.asarray(inputs["batch"]).astype(np.int64)
    src = np.concatenate([ei[0], np.arange(N, dtype=np.int64)])
    dst = np.concatenate([ei[1], np.arange(N, dtype=np.int64)])
    order = np.argsort(dst, kind="stable")
    src, dst = src[order], dst[order]
    deg = np.bincount(dst, minlength=N)
    # edge-balanced core boundaries: cap per-core edges at NW*9*P so 9 subtiles
    # are reachable, while per-core node counts stay within the NLP padded rows
    pref = np.concatenate([[0], np.cumsum(deg - 1)])   # non-self edges
    EMAX = 50120               # ~E_nonself/NCORES: balanced cores, windows fit 8 subtiles
    bounds = [0]
    for c in range(NCORES - 1):
        b = bounds[-1]
        e = int(np.searchsorted(pref, pref[b] + EMAX, side="right") - 1)
        bounds.append(min(e, b + NLP, N))
    bounds.append(N)
    if N - bounds[NCORES - 1] > NLP:
        bounds = list(np.arange(0, N + 1, NLOC))   # fallback: uniform split
    core_lo = np.searchsorted(dst, np.asarray(bounds))

    # per-core balanced assignment of dst nodes to (window, slot)
    raw_plans = []
    for c in range(NCORES):
        lo, hi = bounds[c], bounds[c + 1]
        cntc = hi - lo
        nodes = np.arange(lo, hi)
        d = deg[nodes] - 1          # non-self degree (self-loops ride a
                                    # static per-window DMA, not the gather)
        order_n = np.argsort(-d, kind="stable")
        base = cntc // NW
        cap = np.full(NW, base, np.int64)
        cap[:cntc - base * NW] += 1
        wload = np.zeros(NW, np.int64)
        win_nodes = [[] for _ in range(NW)]
        heap = [(0, w) for w in range(NW)]
        heapq.heapify(heap)
        for i in order_n:
            while True:
                load, w = heapq.heappop(heap)
                if len(win_nodes[w]) < cap[w]:
                    break
            win_nodes[w].append(i)
            wload[w] += d[i]
            if len(win_nodes[w]) < cap[w]:
                heapq.heappush(heap, (int(wload[w]), w))
        raw_plans.append((nodes, win_nodes, wload, d, cap))

    # repair pass: swap nodes between windows to cap every window's load one
    # subtile lower (windows are at node capacity, so only swaps rebalance)
    nat_spw = int(max(int(np.ceil(rp[2].max() / P)) for rp in raw_plans))
    target = (nat_spw - 1) * P
    for c in range(NCORES):
        nodes, win_nodes, wload, d, cap = raw_plans[c]
        if wload.sum() > target * NW:
            continue
        for _ in range(4 * NW):
            w = int(np.argmax(wload))
            if wload[w] <= target:
                break
            done = False
            for i in sorted(win_nodes[w], key=lambda n: -d[n]):
                need = wload[w] - target
                for w2 in np.argsort(wload):
                    if w2 == w:
                        continue
                    cands = [n2 for n2 in win_nodes[int(w2)] if d[i] - d[n2] >= 1
                             and wload[int(w2)] + d[i] - d[n2] <= target]
                    if not cands:
                        continue
                    j = min(cands, key=lambda n2: d[n2]) if need > 1 else \
                        max(cands, key=lambda n2: d[n2])
                    w2 = int(w2)
                    win_nodes[w].remove(i)
                    win_nodes[w2].remove(j)
                    win_nodes[w].append(j)
                    win_nodes[w2].append(i)
                    wload[w] += d[j] - d[i]
                    wload[w2] += d[i] - d[j]
                    done = True
                    break
                if done:
                    break
            if not done:
                break

    plans = []
    for c in range(NCORES):
        nodes, win_nodes, wload, d, cap = raw_plans[c]
        win_of = np.empty(len(nodes), np.int32)
        slot_of = np.empty(len(nodes), np.int32)
        for w in range(NW):
            for s, i in enumerate(win_nodes[w]):
                win_of[i] = w
                slot_of[i] = s
        plans.append((nodes, win_of, slot_of, wload))

    grow_of = np.full(N, -1, np.int64)
    for c, (nodes, win_of, slot_of, _) in enumerate(plans):
        grow_of[nodes] = c * NLP + win_of.astype(np.int64) * WIN + slot_of.astype(np.int64)

    SPW = int(max(int(np.ceil(p[3].max() / P)) for p in plans)) + 1
    NSUB = NW * SPW

    # per core: edges sorted by source table row within each window, p-major
    # flat layout (partition p holds a consecutive sorted run of SPW edges)
    per_core = []
    SPN = SPW - 1               # non-self subtiles per window
    for c in range(NCORES):
        e0, e1 = core_lo[c], core_lo[c + 1]
        es, ed = src[e0:e1], dst[e0:e1]
        nodes, win_of, slot_of, _ = plans[c]
        selfm = es == ed
        es, ed = es[~selfm], ed[~selfm]
        lw = win_of[ed - bounds[c]]
        srow = grow_of[es]
        sl = slot_of[ed - bounds[c]]
        eorder = np.lexsort((srow, lw))
        lw, srow, sl = lw[eorder], srow[eorder], sl[eorder]
        wstart = np.searchsorted(lw, np.arange(NW + 1))
        SRCG = np.zeros((P, NSUB), np.int32)
        SST = np.zeros((P, NW, 2, SPW * P), FP8)
        ghost = np.ones((P, NW), np.float32)
        ghost[slot_of, win_of] = 0.0
        # subtile 0 = self-loops, diagonal by slot (DMA'd, not gathered)
        SST[slot_of, win_of, 0, slot_of] = 1
        SST[slot_of, win_of, 1, slot_of] = 1
        for w in range(NW):
            a, b = wstart[w], wstart[w + 1]
            k = b - a
            assert k <= SPN * P, f"window overflow {k}"
            j = np.arange(k)
            pp = j // SPN          # partition-major: p gets a sorted run
            kk = 1 + j % SPN
            SRCG[pp, w * SPW + kk] = srow[a:b]
            # S: [edge_p partition] x [sub*P + slot]
            SST[pp, w, 0, kk * P + sl[a:b]] = 1
            # ST: [slot partition] x [sub*P + edge_p]
            SST[sl[a:b], w, 1, kk * P + pp] = 1
        per_core.append(dict(SRCG=SRCG,
                             SST=np.ascontiguousarray(SST.reshape(P, NW, 2 * SPW * P)),
                             ghost=ghost))

    gs = np.searchsorted(batch, np.arange(G + 1))
    cnt = (gs[1:] - gs[:-1]).astype(np.float32)
    assert (gs[1:] - gs[:-1]).max() <= PADG
    glists = []
    for c in range(NCORES):
        lo, hi = bounds[c], bounds[c + 1]
        gl = [g for g in range(G) if gs[g] < hi and gs[g + 1] > lo]
        glists.append(gl)
    NG = max(len(gl) for gl in glists)

    for c in range(NCORES):
        nodes, win_of, slot_of, _ = plans[c]
        Sg = np.zeros((P, NW, G), BF16)
        Sg[slot_of, win_of, batch[nodes]] = 1
        per_core[c]["Sg"] = np.ascontiguousarray(Sg.reshape(P, NW * G))
        lo, hi = bounds[c], bounds[c + 1]
        # scatter targets: graph-slot-padded rows (slot s covers [s*PADG, (s+1)*PADG));
        # ghosts land uniquely in the junk tail at NG*PADG+
        gtgt = np.full((NG, 1), G, np.int32)
        slot_of_g = {}
        for i, g in enumerate(glists[c]):
            gtgt[i, 0] = g
            slot_of_g[g] = i
        lpos = np.empty(hi - lo, np.int64)
        for i in range(hi - lo):
            g = batch[lo + i]
            lpos[i] = slot_of_g[g] * PADG + (lo + i - max(gs[g], lo))
        nodepos = np.full((P, NW), -1, np.int64)
        nodepos[slot_of, win_of] = lpos
        gh_p, gh_w = np.where(nodepos < 0)
        nodepos[gh_p, gh_w] = NG * PADG + np.arange(len(gh_p))
        assert nodepos.max() < 32768
        # wrapped int16 for dma_scatter_add: flat j = w*128 + slot
        flat = nodepos.T.reshape(-1)          # [NW*P] with j = w*128+slot
        per_core[c]["npos"] = np.tile(_wrap16(flat, NW * P), (8, 1))
        per_core[c]["gtgt"] = gtgt

    def bf(x):
        return np.ascontiguousarray(np.asarray(x, np.float32)).astype(BF16)

    Wenc = bf(inputs["W_enc"])
    Wg = np.asarray(inputs["Wg"], np.float32)                  # [L, H, H]
    a_s = np.asarray(inputs["att_src"], np.float32)
    a_d = np.asarray(inputs["att_dst"], np.float32)
    Amat = np.zeros((L, H, 2 * HEADS), np.float32)
    for l in range(L):
        for h in range(HEADS):
            Amat[l, 32 * h:32 * h + 32, h] = a_s[l, h]
            Amat[l, 32 * h:32 * h + 32, HEADS + h] = a_d[l, h]
    WgA = np.einsum("lij,ljk->lik", Wg, Amat)                   # [L, H, 16]
    Wfull = np.concatenate([Wg, WgA], axis=2)                   # [L, H, 272]
    WgWa = np.ascontiguousarray(Wfull.reshape(L, 2, P, DHX)).astype(BF16)

    bnp = np.zeros((L + 1, P, 2, 2), np.float32)
    pairs = [(inputs["g_enc"], inputs["be_enc"])] + [(inputs["bn_g"][l], inputs["bn_b"][l]) for l in range(L)]
    for i, (g_, b_) in enumerate(pairs):
        g_, b_ = np.asarray(g_, np.float32), np.asarray(b_, np.float32)
        bnp[i, :, 0, 0], bnp[i, :, 1, 0] = g_[:P], g_[P:]
        bnp[i, :, 0, 1], bnp[i, :, 1, 1] = b_[:P], b_[P:]
    W1 = np.asarray(inputs["W1"], np.float32)
    W1p = np.zeros((7, P, 2 * H), np.float32)
    for kt in range(7):
        r = W1[kt * P:(kt + 1) * P]
        W1p[kt, :r.shape[0]] = r
    W2p = np.ascontiguousarray(np.asarray(inputs["W2"], np.float32)).reshape(4, P, H)
    W3p = np.ascontiguousarray(np.asarray(inputs["W3"], np.float32)).reshape(2, P, NC)
    bn1p = np.zeros((P, 4, 2), np.float32)
    bn1p[:, :, 0] = np.asarray(inputs["g1"], np.float32).reshape(4, P).T
    bn1p[:, :, 1] = np.asarray(inputs["be1"], np.float32).reshape(4, P).T
    bn2p = np.zeros((P, 2, 2), np.float32)
    bn2p[:, :, 0] = np.asarray(inputs["g2"], np.float32).reshape(2, P).T
    bn2p[:, :, 1] = np.asarray(inputs["be2"], np.float32).reshape(2, P).T
    b3 = np.asarray(inputs["b3"], np.float32).reshape(NC, 1)
    gf = np.ascontiguousarray(np.asarray(inputs["graph_features"], np.float32).reshape(G, NGF))
    cntr = (1.0 / cnt).reshape(G, 1).astype(np.float32)

    x = np.asarray(inputs["x"], np.float32)
    in_maps = []
    for c in range(NCORES):
        nodes, win_of, slot_of, _ = plans[c]
        lid = win_of.astype(np.int64) * WIN + slot_of.astype(np.int64)
        xT = np.zeros((F_IN, NLP), np.float32)
        xT[:, lid] = x[nodes].T
        m = dict(per_core[c])
        m.update(xT=xT.astype(BF16), Wenc=Wenc, WgWa=WgWa, bnp=bnp,
                 W1p=W1p, W2p=W2p, W3p=W3p, bn1p=bn1p, bn2p=bn2p, b3=b3,
                 gf=gf, cntr=cntr)
        in_maps.append(m)
    cfg = dict(SPW=SPW, NSUB=NSUB, NG=NG)
    return in_maps, cfg


# ------------------------------------------------------------------ builder
def _build(nc, cfg, debug=False):
    RG = [list(range(NCORES))]
    SPW, NSUB, NG = cfg["SPW"], cfg["NSUB"], cfg["NG"]

    if debug:
        d_dbg_hx = nc.dram_tensor("dbg_hx", [NLP, DROW], BF, kind="ExternalOutput")
        d_dbg_ad = nc.dram_tensor("dbg_ad", [P, NW * HEADS], BF, kind="ExternalOutput")
        d_dbg_zT = nc.dram_tensor("dbg_zT", [P, 2 * NLP], F32, kind="ExternalOutput")
        d_dbg_G = nc.dram_tensor("dbg_G", [P, SPW * DROW], BF, kind="ExternalOutput")

    d_SRCG = nc.dram_tensor("SRCG", [P, NSUB], I32, kind="ExternalInput")
    d_SST = nc.dram_tensor("SST", [P, NW, 2 * SPW * P], F8, kind="ExternalInput")
    d_gh = nc.dram_tensor("ghost", [P, NW], F32, kind="ExternalInput")
    d_np = nc.dram_tensor("npos", [P, NW * P // 16], I16, kind="ExternalInput")
    d_Sg = nc.dram_tensor("Sg", [P, NW * G], BF, kind="ExternalInput")
    d_gtgt = nc.dram_tensor("gtgt", [NG, 1], I32, kind="ExternalInput")
    d_cnt = nc.dram_tensor("cntr", [G, 1], F32, kind="ExternalInput")
    d_xT = nc.dram_tensor("xT", [F_IN, NLP], BF, kind="ExternalInput")
    d_Wenc = nc.dram_tensor("Wenc", [F_IN, H], BF, kind="ExternalInput")
    d_WgWa = nc.dram_tensor("WgWa", [L, 2, P, DHX], BF, kind="ExternalInput")
    d_bnp = nc.dram_tensor("bnp", [L + 1, P, 2, 2], F32, kind="ExternalInput")
    d_W1 = nc.dram_tensor("W1p", [7, P, 2 * H], F32, kind="ExternalInput")
    d_W2 = nc.dram_tensor("W2p", [4, P, H], F32, kind="ExternalInput")
    d_W3 = nc.dram_tensor("W3p", [2, P, NC], F32, kind="ExternalInput")
    d_bn1 = nc.dram_tensor("bn1p", [P, 4, 2], F32, kind="ExternalInput")
    d_bn2 = nc.dram_tensor("bn2p", [P, 2, 2], F32, kind="ExternalInput")
    d_b3 = nc.dram_tensor("b3", [NC, 1], F32, kind="ExternalInput")
    d_gf = nc.dram_tensor("gf", [G, NGF], F32, kind="ExternalInput")
    d_out = nc.dram_tensor("out", [G, NC], F32, kind="ExternalOutput")

    with tile.TileContext(nc, trace_sim=False) as tc:
        with (
            tc.tile_pool(name="sb", bufs=1) as sb,
            tc.tile_pool(name="dr", bufs=2, space="DRAM") as dr,
        ):
            nc.gpsimd.load_library(LIB_MLP)
            idf = sb.tile([P, P], F32)
            make_identity(nc, idf[:])
            idb = sb.tile([P, P], BF)
            make_identity(nc, idb[:])
            eps_sb = sb.tile([P, 1], F32)
            nc.vector.memset(eps_sb[:], EPS)
            # static softmax downscale: exp(e - ln 64). The 1/64 cancels in
            # numerator/denominator; keeps fp16 h*exp products under 65504.
            nl64_sb = sb.tile([P, 1], F32)
            nc.vector.memset(nl64_sb[:], -4.15888308)

            srcg_sb = sb.tile([P, NSUB], I32)
            nc.sync.dma_start(srcg_sb[:], d_SRCG[:])
            ghost_sb = sb.tile([P, NW], F32)
            nc.sync.dma_start(ghost_sb[:], d_gh[:])
            bnp_sb = sb.tile([P, L + 1, 2, 2], F32)
            nc.sync.dma_start(bnp_sb[:], d_bnp[:].rearrange("l p b k -> p l b k"))

            hA = sb.tile([P, 2, NLP], BF)
            hB = sb.tile([P, 2, NLP], BF)
            d_prev = dr.tile([P, 2, NLP], BF, tag="prev", bufs=1)

            # pooling preloads (no deps; issued early so the pooling phase
            # doesn't pay for them)
            sg_sb = sb.tile([P, NW * G], BF)
            nc.sync.dma_start(sg_sb[:], d_Sg[:])
            np_sb = sb.tile([P, NW * P // 16], I16)
            nc.sync.dma_start(np_sb[:], d_np[:])
            NRPL = NG * PADG + NLP
            hf_loc = dr.tile([NRPL, H], BF, tag="hfloc", bufs=1)
            zt0 = sb.tile([P, 2048], BF)
            nc.vector.memset(zt0[:], 0.0)
            for r0 in range(0, NRPL - 1024 + 1, 1024):
                nc.sync.dma_start(
                    hf_loc[r0:r0 + 1024, :].rearrange("(a b) h -> a (b h)", a=P),
                    zt0[:])
            if (NRPL // 1024) * 1024 < NRPL:
                nc.sync.dma_start(
                    hf_loc[NRPL - 1024:NRPL, :].rearrange("(a b) h -> a (b h)", a=P),
                    zt0[:])

            with tc.tile_pool(name="zp", bufs=1) as zp:
                zT = zp.tile([P, 2, NLP], F32)

                def batchnorm_relu(lay, dst_tile, scratch):
                    NCK = 4
                    CK = NLP // NCK
                    stats4 = zp.tile([P, NCK, 4], F32, tag="bnstats4", bufs=2)
                    for ck in range(NCK):
                        c0, c1 = ck * CK, (ck + 1) * CK
                        nc.vector.reduce_sum(stats4[:, ck, 0:1], zT[:, 0, c0:c1], axis=AX.X)
                        nc.vector.reduce_sum(stats4[:, ck, 1:2], zT[:, 1, c0:c1], axis=AX.X)
                        nc.scalar.activation(scratch[:, 0, c0:c1], zT[:, 0, c0:c1], AF.Square,
                                             accum_out=stats4[:, ck, 2:3])
                        nc.scalar.activation(scratch[:, 1, c0:c1], zT[:, 1, c0:c1], AF.Square,
                                             accum_out=stats4[:, ck, 3:4])
                    stats = zp.tile([P, 4], F32, tag="bnstats", bufs=2)
                    nc.vector.tensor_add(stats[:], stats4[:, 0, :], stats4[:, 1, :])
                    nc.vector.tensor_add(stats4[:, 2, :], stats4[:, 2, :], stats4[:, 3, :])
                    nc.vector.tensor_add(stats[:], stats[:], stats4[:, 2, :])
                    sin = dr.tile([P, 4], F32, tag="bnin")
                    sout = dr.tile([P, 4], F32, tag="bnout", addr_space="Shared")
                    nc.sync.dma_start(sin[:], stats[:])
                    nc.gpsimd.collective_compute("AllReduce", ALU.add, replica_groups=RG,
                                                 ins=[sin[:].opt()], outs=[sout[:].opt()])
                    st = zp.tile([P, 4], F32, tag="bnst", bufs=2)
                    nc.sync.dma_start(st[:], sout[:])
                    mu = zp.tile([P, 2], F32, tag="bnmu", bufs=2)
                    nc.vector.tensor_scalar_mul(mu[:], st[:, 0:2], 1.0 / N)
                    var = zp.tile([P, 2], F32, tag="bnvar", bufs=2)
                    nc.vector.tensor_scalar_mul(var[:], st[:, 2:4], 1.0 / N)
                    musq = zp.tile([P, 2], F32, tag="bnmusq", bufs=2)
                    nc.vector.tensor_mul(musq[:], mu[:], mu[:])
                    nc.vector.tensor_sub(var[:], var[:], musq[:])
                    rs = zp.tile([P, 2], F32, tag="bnrs", bufs=2)
                    nc.scalar.activation(rs[:], var[:], AF.Sqrt, bias=eps_sb[:, 0:1])
                    nc.vector.reciprocal(rs[:], rs[:])
                    Sc = zp.tile([P, 2], F32, tag="bnS", bufs=2)
                    nc.vector.tensor_mul(Sc[:], rs[:], bnp_sb[:, lay, :, 0])
                    Bi = zp.tile([P, 2], F32, tag="bnB", bufs=2)
                    nc.vector.tensor_mul(Bi[:], mu[:], Sc[:])
                    nc.vector.tensor_sub(Bi[:], bnp_sb[:, lay, :, 1], Bi[:])
                    for cc in range(4):
                        c0, c1 = cc * (NLP // 4), (cc + 1) * (NLP // 4)
                        for b in range(2):
                            nc.scalar.activation(dst_tile[:, b, c0:c1], zT[:, b, c0:c1], AF.Relu,
                                                 bias=Bi[:, b:b + 1], scale=Sc[:, b:b + 1])

                # ---------------- encoder ----------------
                with (
                    tc.tile_pool(name="encp", bufs=1) as ep,
                    tc.tile_pool(name="psenc", bufs=2, space="PSUM") as ps_enc,
                ):
                    xT_sb = ep.tile([F_IN, NLP], BF)
                    nc.sync.dma_start(xT_sb[:], d_xT[:])
                    wenc_sb = ep.tile([F_IN, H], BF)
                    nc.sync.dma_start(wenc_sb[:], d_Wenc[:])
                    for ch in range(NCH):
                        f0 = ch * 512
                        F = min(512, NLP - f0)
                        for kb in range(2):
                            pz = ps_enc.tile([P, 512], F32, tag="mm")
                            nc.tensor.matmul(pz[:, :F], wenc_sb[:, kb * P:(kb + 1) * P],
                                             xT_sb[:, f0:f0 + F], start=True, stop=True)
                            nc.scalar.activation(zT[:, kb, f0:f0 + F], pz[:, :F], AF.Copy)
                    batchnorm_relu(0, hA, hB)

                # ---------------- GAT layers ----------------
                with (
                    tc.tile_pool(name="edge", bufs=1) as eb,
                    tc.tile_pool(name="pspj", bufs=2, space="PSUM") as ps_pj,
                    tc.tile_pool(name="pswin", bufs=2, space="PSUM") as ps_win,
                    tc.tile_pool(name="pstr", bufs=2, space="PSUM") as ps_tr,
                ):
                    adst = eb.tile([P, NW, HEADS], BF, tag="adst", bufs=2)
                    for l in range(L):
                        hin = hA if l % 2 == 0 else hB
                        hout = hB if l % 2 == 0 else hA
                        wg_sb = eb.tile([P, 2, DHX], BF, tag="wg", bufs=2)
                        nc.sync.dma_start(wg_sb[:], d_WgWa[l].rearrange("t p k -> p t k"))

                        # fused projection: node-major [128, 272] per window
                        hx_loc = dr.tile([NLP, DROW], BF, tag="hxloc")
                        for w in range(NW):
                            n0 = w * WIN
                            pz = ps_pj.tile([P, DHX], F32, tag="pj")
                            for jt in range(2):
                                nc.tensor.matmul(pz[:], hin[:, jt, n0:n0 + P],
                                                 wg_sb[:, jt, :], start=(jt == 0), stop=(jt == 1))
                            hxw = eb.tile([P, DHX], BF, tag="hxw", bufs=3)
                            nc.scalar.activation(hxw[:], pz[:], AF.Copy)
                            nc.vector.tensor_copy(adst[:, w, :], hxw[:, H + HEADS:DHX])
                            nc.sync.dma_start(hx_loc[n0:n0 + P, :], hxw[:, 0:DROW])

                        hx_full = dr.tile([NCORES, NLP, DROW], BF, tag="hxfull", addr_space="Shared")
                        nc.gpsimd.collective_compute("AllGather", ALU.bypass, replica_groups=RG,
                                                     ins=[hx_loc[:].opt()],
                                                     outs=[hx_full[:].opt()])
                        tab = hx_full[:].rearrange("c n d -> (c n) d")

                        for w in range(NW):
                            sst = eb.tile([P, 2, SPW * P], F8, tag="sst", bufs=6)
                            nc.sync.dma_start(sst[:].rearrange("p a b -> p (a b)"), d_SST[:, w])
                            Gt = eb.tile([P, SPW, DROW], BF, tag="G", bufs=6)
                            # subtile 0 = self-loops: the window's own local
                            # rows, one contiguous HWDGE DMA (no AG dep, no Q7)
                            nc.sync.dma_start(Gt[:, 0, :], hx_loc[w * WIN:(w + 1) * WIN, :])
                            for k in range(1, SPW):
                                nc.gpsimd.indirect_dma_start(
                                    out=Gt[:, k, :], out_offset=None,
                                    in_=tab,
                                    in_offset=bass.IndirectOffsetOnAxis(
                                        ap=srcg_sb[:, w * SPW + k:w * SPW + k + 1], axis=0))
                            if debug and l == 0 and w == 0:
                                nc.sync.dma_start(d_dbg_G[:], Gt[:].rearrange("p j d -> p (j d)"))
                            T = ps_win.tile([P, 264 + SPW * HEADS], F32, tag="win")
                            for k in range(SPW):
                                nc.tensor.matmul(T[:, 264 + k * HEADS:264 + (k + 1) * HEADS],
                                                 sst[:, 1, k * P:(k + 1) * P],
                                                 adst[:, w, :], start=True, stop=True)
                            asc = eb.tile([P, SPW, HEADS], F32, tag="asc", bufs=3)
                            nc.vector.tensor_copy(asc[:], Gt[:, :, H:H + HEADS])
                            et = eb.tile([P, SPW, HEADS], F32, tag="et", bufs=3)
                            nc.vector.tensor_add(
                                et[:], asc[:],
                                T[:, 264:264 + SPW * HEADS].rearrange("p (j h) -> p j h", h=HEADS))
                            et3 = eb.tile([P, SPW, HEADS], F32, tag="et3", bufs=3)
                            nc.vector.tensor_scalar_mul(et3[:], et[:], 0.2)
                            nc.vector.tensor_max(et3[:], et3[:], et[:])
                            etb = eb.tile([P, SPW, HEADS], BF, tag="etb", bufs=2)
                            nc.scalar.activation(etb[:], et3[:], AF.Exp, bias=nl64_sb[:, 0:1])
                            # expanded exp on the scalar engine keeps the big
                            # DVE multiply fully contiguous
                            exb = eb.tile([P, SPW, HEADS, HD], BF, tag="exb", bufs=2)
                            nc.scalar.activation(
                                exb[:], et3[:, :, :, None].to_broadcast([P, SPW, HEADS, HD]),
                                AF.Exp, bias=nl64_sb[:, 0:1])
                            # exp into the a_src slot: feeds the denominator
                            # columns of the scatter-add matmul
                            nc.vector.tensor_copy(Gt[:, :, H:H + HEADS], etb[:])
                            gv = Gt[:, :, 0:H].rearrange("p j (h d) -> p j h d", h=HEADS)
                            nc.vector.tensor_mul(gv[:], gv[:], exb[:])
                            for k in range(SPW):
                                nc.tensor.matmul(T[:, 0:264], sst[:, 0, k * P:(k + 1) * P],
                                                 Gt[:, k, 0:264],
                                                 start=(k == 0), stop=(k == SPW - 1))
                            den = eb.tile([P, HEADS], F32, tag="den", bufs=3)
                            nc.vector.tensor_scalar(den[:], T[:, 256:264],
                                                    scalar1=ghost_sb[:, w:w + 1],
                                                    scalar2=None, op0=ALU.add)
                            nc.vector.reciprocal(den[:], den[:])
                            rcx = eb.tile([P, HEADS, HD], F32, tag="rcx", bufs=3)
                            nc.scalar.activation(
                                rcx[:], den[:, :, None].to_broadcast([P, HEADS, HD]), AF.Copy)
                            zw = eb.tile([P, H], F32, tag="zw", bufs=3)
                            nc.vector.tensor_mul(
                                zw[:].rearrange("p (h d) -> p h d", h=HEADS),
                                T[:, 0:H].rearrange("p (h d) -> p h d", h=HEADS),
                                rcx[:])
                            for b in range(2):
                                pt = ps_tr.tile([P, P], F32, tag="trf")
                                nc.tensor.transpose(out=pt[:], in_=zw[:, b * P:(b + 1) * P],
                                                    identity=idf[:])
                                if b == 0:
                                    nc.scalar.activation(zT[:, b, w * WIN:w * WIN + P], pt[:], AF.Copy)
                                else:
                                    nc.vector.tensor_copy(zT[:, b, w * WIN:w * WIN + P], pt[:])

                        if debug and l == 0:
                            nc.sync.dma_start(d_dbg_hx[:], hx_loc[:])
                            nc.sync.dma_start(d_dbg_ad[:], adst[:].rearrange("p w h -> p (w h)"))
                            nc.sync.dma_start(d_dbg_zT[:], zT[:].rearrange("p b n -> p (b n)"))
                        batchnorm_relu(l + 1, hout, hin)
                        if l == 1:
                            nc.sync.dma_start(d_prev[:], hout[:])
                        if l == 2:
                            for b in range(2):
                                for cc in range(7):
                                    c0 = cc * 896
                                    cw = min(896, NLP - c0)
                                    psc = eb.tile([P, 896], BF, tag="prevc", bufs=4)
                                    nc.sync.dma_start(psc[:, :cw], d_prev[:, b, c0:c0 + cw])
                                    nc.vector.tensor_add(hout[:, b, c0:c0 + cw], hout[:, b, c0:c0 + cw],
                                                         psc[:, :cw])

            # ---------------- pooling ----------------
            hfin = hA if L % 2 == 0 else hB
            with (
                tc.tile_pool(name="poolp", bufs=1) as pb,
                tc.tile_pool(name="psmm", bufs=2, space="PSUM") as ps_mm,
                tc.tile_pool(name="pstr2", bufs=2, space="PSUM") as ps_tr,
            ):
                hstage = pb.tile([P, NW, H], BF)
                pp0 = ps_mm.tile([G, H], F32, tag="mm")
                for w in range(NW):
                    n0 = w * WIN
                    for b in range(2):
                        pt = ps_tr.tile([P, P], BF, tag="trb", bufs=2)
                        nc.tensor.transpose(out=pt[:], in_=hfin[:, b, n0:n0 + P], identity=idb[:])
                        nc.vector.tensor_copy(hstage[:, w, b * P:(b + 1) * P], pt[:])
                    nc.tensor.matmul(pp0[:], sg_sb[:, w * G:(w + 1) * G], hstage[:, w, :],
                                     start=(w == 0), stop=(w == NW - 1))

                # per-graph max: scatter-add node rows into the zeroed
                # graph-slot-padded buffer (h >= 0 post-ReLU, so 0-pads are
                # neutral for max), then dma-transpose each slot and reduce
                HWN = 24 * P
                nc.gpsimd.dma_scatter_add(
                    hf_loc[:], hstage[:, 0:24, :], np_sb[:, 0:HWN // 16],
                    HWN, HWN, H)
                nc.gpsimd.dma_scatter_add(
                    hf_loc[:], hstage[:, 24:NW, :], np_sb[:, HWN // 16:NW * P // 16],
                    (NW - 24) * P, (NW - 24) * P, H)

                pmax = pb.tile([P, 2, NG], F32)
                for i in range(NG):
                    gt = pb.tile([P, 2, PADG], BF, tag="gt", bufs=2)
                    for b in range(2):
                        nc.sync.dma_start_transpose(
                            gt[:, b, :], hf_loc[i * PADG:(i + 1) * PADG, b * P:(b + 1) * P])
                    for b in range(2):
                        nc.vector.reduce_max(pmax[:, b, i:i + 1], gt[:, b, :], axis=AX.X)

                # rows: [G, 2H] = [sum | max]; combine across cores with one AG
                pmax_rows = pb.tile([P, H], F32)
                for b in range(2):
                    pt = ps_tr.tile([P, P], F32, tag="trf")
                    nc.tensor.transpose(out=pt[0:NG, 0:P], in_=pmax[:, b, :], identity=idf[:])
                    nc.scalar.activation(pmax_rows[0:NG, b * P:(b + 1) * P], pt[0:NG, 0:P], AF.Copy)

                pin_sb = pb.tile([G, 2 * H], F32)
                nc.scalar.activation(pin_sb[:, 0:H], pp0[:], AF.Copy)
                nc.vector.memset(pin_sb[:, H:2 * H], 0.0)
                pin = dr.tile([G + 1, 2 * H], F32, tag="pin")
                nc.sync.dma_start(pin[0:G, :], pin_sb[:])
                gtgt_sb = pb.tile([NG, 1], I32)
                nc.sync.dma_start(gtgt_sb[:], d_gtgt[:])
                nc.gpsimd.indirect_dma_start(
                    out=pin[:], out_offset=bass.IndirectOffsetOnAxis(ap=gtgt_sb[:], axis=0),
                    in_=pmax_rows[0:NG, :], in_offset=None, element_offset=H)

                pfull_d = dr.tile([NCORES, G, 2 * H], F32, tag="pfull", addr_space="Shared")
                nc.gpsimd.collective_compute("AllGather", ALU.bypass, replica_groups=RG,
                                             ins=[pin[0:G, :].opt()], outs=[pfull_d[:].opt()])
                pf8 = pb.tile([G, NCORES, 2 * H], F32)
                nc.sync.dma_start(pf8[:], pfull_d[:].rearrange("c g h -> g c h"))
                for cc in range(1, NCORES):
                    nc.vector.tensor_add(pf8[:, 0, 0:H], pf8[:, 0, 0:H], pf8[:, cc, 0:H])
                    nc.vector.tensor_max(pf8[:, 0, H:2 * H], pf8[:, 0, H:2 * H], pf8[:, cc, H:2 * H])

                # pooled [G, 800] = [mean | max | sum | gf]
                pooled = pb.tile([G, 3 * H + NGF], F32)
                cnt_sb = pb.tile([G, 1], F32)
                nc.sync.dma_start(cnt_sb[:], d_cnt[:])
                nc.vector.tensor_scalar(pooled[:, 0:H], pf8[:, 0, 0:H], scalar1=cnt_sb[:],
                                        scalar2=None, op0=ALU.mult)
                nc.vector.tensor_copy(pooled[:, H:2 * H], pf8[:, 0, H:2 * H])
                nc.vector.tensor_copy(pooled[:, 2 * H:3 * H], pf8[:, 0, 0:H])
                nc.sync.dma_start(pooled[:, 3 * H:], d_gf[:])

                # ---------------- classifier (replicated) ----------------
                pT = pb.tile([P, 7, G], F32)
                nc.vector.memset(pT[:], 0)
                for t in range(7):
                    w_ = min(P, 3 * H + NGF - t * P)
                    pt = ps_mm.tile([P, P], F32, tag="mm")
                    nc.tensor.transpose(out=pt[0:w_, 0:G], in_=pooled[:, t * P:t * P + w_],
                                        identity=idf[0:G, 0:G])
                    nc.scalar.activation(pT[:w_, t, :], pt[:w_, 0:G], AF.Copy)

                w1_sb = pb.tile([P, 7, 2 * H], F32)
                nc.sync.dma_start(w1_sb[:], d_W1[:].rearrange("t p k -> p t k"))
                bn1_sb = pb.tile([P, 4, 2], F32)
                nc.sync.dma_start(bn1_sb[:], d_bn1[:])
                z1 = pb.tile([P, 4, G], F32)

                def mlp_bn(zt, nblk, bnsb, ngraph=G):
                    for b in range(nblk):
                        s_ = pb.tile([P, 1], F32, tag="cbs", bufs=2)
                        nc.vector.reduce_sum(s_[:], zt[:, b, :], axis=AX.X)
                        sqt = pb.tile([P, G], F32, tag="cbsq", bufs=2)
                        q_ = pb.tile([P, 1], F32, tag="cbq", bufs=2)
                        nc.scalar.activation(sqt[:], zt[:, b, :], AF.Square, accum_out=q_[:])
                        mu = pb.tile([P, 1], F32, tag="cbmu", bufs=2)
                        nc.vector.tensor_scalar_mul(mu[:], s_[:], 1.0 / ngraph)
                        var = pb.tile([P, 1], F32, tag="cbvar", bufs=2)
                        nc.vector.tensor_scalar_mul(var[:], q_[:], 1.0 / ngraph)
                        ms = pb.tile([P, 1], F32, tag="cbms", bufs=2)
                        nc.vector.tensor_mul(ms[:], mu[:], mu[:])
                        nc.vector.tensor_sub(var[:], var[:], ms[:])
                        rs = pb.tile([P, 1], F32, tag="cbrs", bufs=2)
                        nc.scalar.activation(rs[:], var[:], AF.Sqrt, bias=eps_sb[:, 0:1])
                        nc.vector.reciprocal(rs[:], rs[:])
                        Sc = pb.tile([P, 1], F32, tag="cbS", bufs=2)
                        nc.vector.tensor_mul(Sc[:], rs[:], bnsb[:, b, 0:1])
                        Bi = pb.tile([P, 1], F32, tag="cbB", bufs=2)
                        nc.vector.tensor_mul(Bi[:], mu[:], Sc[:])
                        nc.vector.tensor_sub(Bi[:], bnsb[:, b, 1:2], Bi[:])
                        nc.scalar.activation(zt[:, b, :], zt[:, b, :], AF.Relu,
                                             bias=Bi[:], scale=Sc[:])

                for mb in range(4):
                    pz = ps_mm.tile([P, 512], F32, tag="mm")
                    for kt in range(7):
                        nc.tensor.matmul(pz[:, 0:G], w1_sb[:, kt, mb * P:(mb + 1) * P],
                                         pT[:, kt, :], start=(kt == 0), stop=(kt == 6))
                    nc.scalar.activation(z1[:, mb, :], pz[:, 0:G], AF.Copy)
                mlp_bn(z1, 4, bn1_sb)

                w2_sb = pb.tile([P, 4, H], F32)
                nc.sync.dma_start(w2_sb[:], d_W2[:].rearrange("t p k -> p t k"))
                bn2_sb = pb.tile([P, 2, 2], F32)
                nc.sync.dma_start(bn2_sb[:], d_bn2[:])
                z2 = pb.tile([P, 2, G], F32)
                for mb in range(2):
                    pz = ps_mm.tile([P, 512], F32, tag="mm")
                    for kt in range(4):
                        nc.tensor.matmul(pz[:, 0:G], w2_sb[:, kt, mb * P:(mb + 1) * P],
                                         z1[:, kt, :], start=(kt == 0), stop=(kt == 3))
                    nc.scalar.activation(z2[:, mb, :], pz[:, 0:G], AF.Copy)
                mlp_bn(z2, 2, bn2_sb)

                w3_sb = pb.tile([P, 2, NC], F32)
                nc.sync.dma_start(w3_sb[:], d_W3[:].rearrange("t p k -> p t k"))
                b3_sb = pb.tile([NC, 1], F32)
                nc.sync.dma_start(b3_sb[:], d_b3[:])
                pz3 = ps_mm.tile([P, 512], F32, tag="mm")
                for kt in range(2):
                    nc.tensor.matmul(pz3[0:NC, 0:G], w3_sb[:, kt, :], z2[:, kt, :],
                                     start=(kt == 0), stop=(kt == 1))
                z3 = pb.tile([NC, G], F32)
                nc.scalar.activation(z3[:], pz3[0:NC, 0:G], AF.Identity, bias=b3_sb[:, 0:1])
                nc.sync.dma_start(d_out[:].rearrange("g c -> c g"), z3[:])
    return nc


_CACHE = {}


def _get_compiled(cfg, debug=False):
    key = (cfg["SPW"], cfg["NSUB"], cfg["NG"], debug)
    if key not in _CACHE:
        nc = bacc.Bacc("TRN2", target_bir_lowering=False, debug=False,
                       num_devices=NCORES, dynamic_dma_scratch_size=24576)
        _build(nc, cfg, debug=debug)
        nc.compile()
        _CACHE[key] = nc
    return _CACHE[key]


def kernel(debug=False, _want_results=False, **inputs):
    in_maps, cfg = _prep(inputs)
    nc = _get_compiled(cfg)
    res = run_bass_kernel_spmd(nc, in_maps, core_ids=list(range(NCORES)))
    out = np.asarray(res.results[0]["out"], np.float32)
    if _want_results:
        return out, res
    return out


# revision 26
# speedup vs baseline: 1.0918x; 1.0020x over previous
"""AddressAwareGNN (4-layer GAT + concat pooling + MLP) on 8 Trainium2 cores.

Sharding: nodes/edges partitioned by destination-node range (graph parallel).
Per layer: a fused projection emits node-major rows [256 feat | 8 a_src]
straight from the matmul (attention vectors folded into the weights on host,
so no separate attention matmul and no feature-major->node-major transposes),
AllGather of the 528B rows, then per-window edge aggregation: one SWDGE
indirect gather per 128-edge subtile (edges sorted by source row for HBM
locality), a_dst distributed via one-hot fp8 matmuls, exp on the scalar engine
(with a static -ln64 bias so fp16 h*exp products cannot overflow; the 1/64
cancels in the softmax), and one-hot fp8 matmuls for the segment-softmax
scatter-add. All 16-bit tensors are fp16 (not bf16) for the extra mantissa.
BatchNorm batch-stats via a small AllReduce, stats/apply chunked so they
overlap the aggregation tail and the next projection. Pooling: segment-sum via
one-hot matmul, per-graph max via dma_scatter_add into a zeroed graph-padded
buffer + transposing loads; classifier replicated on all cores.
"""
import os
import sys

sys.path.insert(0, "/opt/trn_rl_repo")

import heapq
import numpy as np
import ml_dtypes

import concourse.bass as bass
import concourse.mybir as mybir
import concourse.tile as tile
from concourse import bacc
from concourse.bass_utils import run_bass_kernel_spmd
from concourse.library_config import mlp as LIB_MLP
from concourse.masks import make_identity

BF16 = np.float16          # 16-bit activations: fp16 (more mantissa than bf16)
FP8 = ml_dtypes.float8_e4m3
F32 = mybir.dt.float32
BF = mybir.dt.float16
F8 = mybir.dt.float8e4
I32 = mybir.dt.int32
I16 = mybir.dt.int16
AF = mybir.ActivationFunctionType
ALU = mybir.AluOpType
AX = mybir.AxisListType

N, F_IN, H, HEADS, HD, L, G, NGF, NC = 50000, 64, 256, 8, 32, 4, 64, 32, 2
EPS = 1e-5
NCORES = 8
NLOC = N // NCORES          # 6250
NW = 49
WIN = 128
NLP = NW * WIN              # 6272 padded local rows
NGLOB = NLP * NCORES        # 50176
DROW = 264                  # table row: [256 feat | 8 a_src]
DHX = H + 2 * HEADS         # 272 meaningful columns
P = 128
PADG = 1024                 # max nodes per graph (gather padding)
NCH = (NLP + 511) // 512    # 13 feature-major column chunks


# ------------------------------------------------------------------ host prep
def _wrap16(idxs, cap):
    """Pack cap int16 indices into the [16, cap//16] column-major wrap."""
    arr = np.zeros((16, cap // 16), np.int16)
    j = np.arange(len(idxs))
    arr[j % 16, j // 16] = idxs
    return arr


def _prep(inputs):
    ei = np.asarray(inputs["edge_index"]).astype(np.int64)
    batch = np.asarray(inputs["batch"]).astype(np.int64)
    src = np.concatenate([ei[0], np.arange(N, dtype=np.int64)])
    dst = np.concatenate([ei[1], np.arange(N, dtype=np.int64)])
    # flag only the APPENDED self-loops (natural src==dst edges in the input
    # must stay in the gathered path — each is a separate softmax term)
    isloop = np.concatenate([np.zeros(ei.shape[1], bool), np.ones(N, bool)])
    order = np.argsort(dst, kind="stable")
    src, dst, isloop = src[order], dst[order], isloop[order]
    deg = np.bincount(dst, minlength=N)
    # edge-balanced core boundaries: cap per-core edges at NW*9*P so 9 subtiles
    # are reachable, while per-core node counts stay within the NLP padded rows
    pref = np.concatenate([[0], np.cumsum(deg - 1)])   # non-self edges
    EMAX = 50120               # ~E_nonself/NCORES: balanced cores, windows fit 8 subtiles
    bounds = [0]
    for c in range(NCORES - 1):
        b = bounds[-1]
        e = int(np.searchsorted(pref, pref[b] + EMAX, side="right") - 1)
        bounds.append(min(e, b + NLP, N))
    bounds.append(N)
    if N - bounds[NCORES - 1] > NLP:
        bounds = list(np.arange(0, N + 1, NLOC))   # fallback: uniform split
    core_lo = np.searchsorted(dst, np.asarray(bounds))

    # per-core balanced assignment of dst nodes to (window, slot)
    raw_plans = []
    for c in range(NCORES):
        lo, hi = bounds[c], bounds[c + 1]
        cntc = hi - lo
        nodes = np.arange(lo, hi)
        d = deg[nodes] - 1          # non-self degree (self-loops ride a
                                    # static per-window DMA, not the gather)
        order_n = np.argsort(-d, kind="stable")
        base = cntc // NW
        cap = np.full(NW, base, np.int64)
        cap[:cntc - base * NW] += 1
        wload = np.zeros(NW, np.int64)
        win_nodes = [[] for _ in range(NW)]
        heap = [(0, w) for w in range(NW)]
        heapq.heapify(heap)
        for i in order_n:
            while True:
                load, w = heapq.heappop(heap)
                if len(win_nodes[w]) < cap[w]:
                    break
            win_nodes[w].append(i)
            wload[w] += d[i]
            if len(win_nodes[w]) < cap[w]:
                heapq.heappush(heap, (int(wload[w]), w))
        raw_plans.append((nodes, win_nodes, wload, d, cap))

    # repair pass: swap nodes between windows to cap every window's load one
    # subtile lower (windows are at node capacity, so only swaps rebalance)
    nat_spw = int(max(int(np.ceil(rp[2].max() / P)) for rp in raw_plans))
    target = (nat_spw - 1) * P
    for c in range(NCORES):
        nodes, win_nodes, wload, d, cap = raw_plans[c]
        if wload.sum() > target * NW:
            continue
        for _ in range(4 * NW):
            w = int(np.argmax(wload))
            if wload[w] <= target:
                break
            done = False
            for i in sorted(win_nodes[w], key=lambda n: -d[n]):
                need = wload[w] - target
                for w2 in np.argsort(wload):
                    if w2 == w:
                        continue
                    cands = [n2 for n2 in win_nodes[int(w2)] if d[i] - d[n2] >= 1
                             and wload[int(w2)] + d[i] - d[n2] <= target]
                    if not cands:
                        continue
                    j = min(cands, key=lambda n2: d[n2]) if need > 1 else \
                        max(cands, key=lambda n2: d[n2])
                    w2 = int(w2)
                    win_nodes[w].remove(i)
                    win_nodes[w2].remove(j)
                    win_nodes[w].append(j)
                    win_nodes[w2].append(i)
                    wload[w] += d[j] - d[i]
                    wload[w2] += d[i] - d[j]
                    done = True
                    break
                if done:
                    break
            if not done:
                break

    plans = []
    for c in range(NCORES):
        nodes, win_nodes, wload, d, cap = raw_plans[c]
        win_of = np.empty(len(nodes), np.int32)
        slot_of = np.empty(len(nodes), np.int32)
        for w in range(NW):
            for s, i in enumerate(win_nodes[w]):
                win_of[i] = w
                slot_of[i] = s
        plans.append((nodes, win_of, slot_of, wload))

    grow_of = np.full(N, -1, np.int64)
    for c, (nodes, win_of, slot_of, _) in enumerate(plans):
        grow_of[nodes] = c * NLP + win_of.astype(np.int64) * WIN + slot_of.astype(np.int64)

    SPW = int(max(int(np.ceil(p[3].max() / P)) for p in plans)) + 1
    NSUB = NW * SPW

    # per core: edges sorted by source table row within each window, p-major
    # flat layout (partition p holds a consecutive sorted run of SPW edges)
    per_core = []
    SPN = SPW - 1               # non-self subtiles per window
    for c in range(NCORES):
        e0, e1 = core_lo[c], core_lo[c + 1]
        es, ed = src[e0:e1], dst[e0:e1]
        nodes, win_of, slot_of, _ = plans[c]
        selfm = isloop[e0:e1]
        es, ed = es[~selfm], ed[~selfm]
        lw = win_of[ed - bounds[c]]
        srow = grow_of[es]
        sl = slot_of[ed - bounds[c]]
        eorder = np.lexsort((srow, lw))
        lw, srow, sl = lw[eorder], srow[eorder], sl[eorder]
        wstart = np.searchsorted(lw, np.arange(NW + 1))
        SRCG = np.zeros((P, NSUB), np.int32)
        SST = np.zeros((P, NW, 2, SPW * P), FP8)
        ghost = np.ones((P, NW), np.float32)
        ghost[slot_of, win_of] = 0.0
        # subtile 0 = self-loops, diagonal by slot (DMA'd, not gathered)
        SST[slot_of, win_of, 0, slot_of] = 1
        SST[slot_of, win_of, 1, slot_of] = 1
        for w in range(NW):
            a, b = wstart[w], wstart[w + 1]
            k = b - a
            assert k <= SPN * P, f"window overflow {k}"
            j = np.arange(k)
            pp = j // SPN          # partition-major: p gets a sorted run
            kk = 1 + j % SPN
            SRCG[pp, w * SPW + kk] = srow[a:b]
            # S: [edge_p partition] x [sub*P + slot]
            SST[pp, w, 0, kk * P + sl[a:b]] = 1
            # ST: [slot partition] x [sub*P + edge_p]
            SST[sl[a:b], w, 1, kk * P + pp] = 1
        per_core.append(dict(SRCG=SRCG,
                             SST=np.ascontiguousarray(SST.reshape(P, NW, 2 * SPW * P)),
                             ghost=ghost))

    gs = np.searchsorted(batch, np.arange(G + 1))
    cnt = (gs[1:] - gs[:-1]).astype(np.float32)
    assert (gs[1:] - gs[:-1]).max() <= PADG
    glists = []
    for c in range(NCORES):
        lo, hi = bounds[c], bounds[c + 1]
        gl = [g for g in range(G) if gs[g] < hi and gs[g + 1] > lo]
        glists.append(gl)
    NG = max(len(gl) for gl in glists)

    for c in range(NCORES):
        nodes, win_of, slot_of, _ = plans[c]
        Sg = np.zeros((P, NW, G), BF16)
        Sg[slot_of, win_of, batch[nodes]] = 1
        per_core[c]["Sg"] = np.ascontiguousarray(Sg.reshape(P, NW * G))
        lo, hi = bounds[c], bounds[c + 1]
        # scatter targets: graph-slot-padded rows (slot s covers [s*PADG, (s+1)*PADG));
        # ghosts land uniquely in the junk tail at NG*PADG+
        gtgt = np.full((NG, 1), G, np.int32)
        slot_of_g = {}
        for i, g in enumerate(glists[c]):
            gtgt[i, 0] = g
            slot_of_g[g] = i
        lpos = np.empty(hi - lo, np.int64)
        for i in range(hi - lo):
            g = batch[lo + i]
            lpos[i] = slot_of_g[g] * PADG + (lo + i - max(gs[g], lo))
        nodepos = np.full((P, NW), -1, np.int64)
        nodepos[slot_of, win_of] = lpos
        gh_p, gh_w = np.where(nodepos < 0)
        nodepos[gh_p, gh_w] = NG * PADG + np.arange(len(gh_p))
        assert nodepos.max() < 32768
        # wrapped int16 for dma_scatter_add: flat j = w*128 + slot
        flat = nodepos.T.reshape(-1)          # [NW*P] with j = w*128+slot
        per_core[c]["npos"] = np.tile(_wrap16(flat, NW * P), (8, 1))
        per_core[c]["gtgt"] = gtgt

    def bf(x):
        return np.ascontiguousarray(np.asarray(x, np.float32)).astype(BF16)

    Wenc = bf(inputs["W_enc"])
    Wg = np.asarray(inputs["Wg"], np.float32)                  # [L, H, H]
    a_s = np.asarray(inputs["att_src"], np.float32)
    a_d = np.asarray(inputs["att_dst"], np.float32)
    Amat = np.zeros((L, H, 2 * HEADS), np.float32)
    for l in range(L):
        for h in range(HEADS):
            Amat[l, 32 * h:32 * h + 32, h] = a_s[l, h]
            Amat[l, 32 * h:32 * h + 32, HEADS + h] = a_d[l, h]
    WgA = np.einsum("lij,ljk->lik", Wg, Amat)                   # [L, H, 16]
    Wfull = np.concatenate([Wg, WgA], axis=2)                   # [L, H, 272]
    WgWa = np.ascontiguousarray(Wfull.reshape(L, 2, P, DHX)).astype(BF16)

    bnp = np.zeros((L + 1, P, 2, 2), np.float32)
    pairs = [(inputs["g_enc"], inputs["be_enc"])] + [(inputs["bn_g"][l], inputs["bn_b"][l]) for l in range(L)]
    for i, (g_, b_) in enumerate(pairs):
        g_, b_ = np.asarray(g_, np.float32), np.asarray(b_, np.float32)
        bnp[i, :, 0, 0], bnp[i, :, 1, 0] = g_[:P], g_[P:]
        bnp[i, :, 0, 1], bnp[i, :, 1, 1] = b_[:P], b_[P:]
    W1 = np.asarray(inputs["W1"], np.float32)
    W1p = np.zeros((7, P, 2 * H), np.float32)
    for kt in range(7):
        r = W1[kt * P:(kt + 1) * P]
        W1p[kt, :r.shape[0]] = r
    W2p = np.ascontiguousarray(np.asarray(inputs["W2"], np.float32)).reshape(4, P, H)
    W3p = np.ascontiguousarray(np.asarray(inputs["W3"], np.float32)).reshape(2, P, NC)
    bn1p = np.zeros((P, 4, 2), np.float32)
    bn1p[:, :, 0] = np.asarray(inputs["g1"], np.float32).reshape(4, P).T
    bn1p[:, :, 1] = np.asarray(inputs["be1"], np.float32).reshape(4, P).T
    bn2p = np.zeros((P, 2, 2), np.float32)
    bn2p[:, :, 0] = np.asarray(inputs["g2"], np.float32).reshape(2, P).T
    bn2p[:, :, 1] = np.asarray(inputs["be2"], np.float32).reshape(2, P).T
    b3 = np.asarray(inputs["b3"], np.float32).reshape(NC, 1)
    gf = np.ascontiguousarray(np.asarray(inputs["graph_features"], np.float32).reshape(G, NGF))
    cntr = (1.0 / cnt).reshape(G, 1).astype(np.float32)

    x = np.asarray(inputs["x"], np.float32)
    in_maps = []
    for c in range(NCORES):
        nodes, win_of, slot_of, _ = plans[c]
        lid = win_of.astype(np.int64) * WIN + slot_of.astype(np.int64)
        xT = np.zeros((F_IN, NLP), np.float32)
        xT[:, lid] = x[nodes].T
        m = dict(per_core[c])
        m.update(xT=xT.astype(BF16), Wenc=Wenc, WgWa=WgWa, bnp=bnp,
                 W1p=W1p, W2p=W2p, W3p=W3p, bn1p=bn1p, bn2p=bn2p, b3=b3,
                 gf=gf, cntr=cntr)
        in_maps.append(m)
    cfg = dict(SPW=SPW, NSUB=NSUB, NG=NG)
    return in_maps, cfg


# ------------------------------------------------------------------ builder
def _build(nc, cfg, debug=False):
    RG = [list(range(NCORES))]
    SPW, NSUB, NG = cfg["SPW"], cfg["NSUB"], cfg["NG"]

    if debug:
        d_dbg_hx = nc.dram_tensor("dbg_hx", [NLP, DROW], BF, kind="ExternalOutput")
        d_dbg_ad = nc.dram_tensor("dbg_ad", [P, NW * HEADS], BF, kind="ExternalOutput")
        d_dbg_zT = nc.dram_tensor("dbg_zT", [P, 2 * NLP], F32, kind="ExternalOutput")
        d_dbg_G = nc.dram_tensor("dbg_G", [P, SPW * DROW], BF, kind="ExternalOutput")

    d_SRCG = nc.dram_tensor("SRCG", [P, NSUB], I32, kind="ExternalInput")
    d_SST = nc.dram_tensor("SST", [P, NW, 2 * SPW * P], F8, kind="ExternalInput")
    d_gh = nc.dram_tensor("ghost", [P, NW], F32, kind="ExternalInput")
    d_np = nc.dram_tensor("npos", [P, NW * P // 16], I16, kind="ExternalInput")
    d_Sg = nc.dram_tensor("Sg", [P, NW * G], BF, kind="ExternalInput")
    d_gtgt = nc.dram_tensor("gtgt", [NG, 1], I32, kind="ExternalInput")
    d_cnt = nc.dram_tensor("cntr", [G, 1], F32, kind="ExternalInput")
    d_xT = nc.dram_tensor("xT", [F_IN, NLP], BF, kind="ExternalInput")
    d_Wenc = nc.dram_tensor("Wenc", [F_IN, H], BF, kind="ExternalInput")
    d_WgWa = nc.dram_tensor("WgWa", [L, 2, P, DHX], BF, kind="ExternalInput")
    d_bnp = nc.dram_tensor("bnp", [L + 1, P, 2, 2], F32, kind="ExternalInput")
    d_W1 = nc.dram_tensor("W1p", [7, P, 2 * H], F32, kind="ExternalInput")
    d_W2 = nc.dram_tensor("W2p", [4, P, H], F32, kind="ExternalInput")
    d_W3 = nc.dram_tensor("W3p", [2, P, NC], F32, kind="ExternalInput")
    d_bn1 = nc.dram_tensor("bn1p", [P, 4, 2], F32, kind="ExternalInput")
    d_bn2 = nc.dram_tensor("bn2p", [P, 2, 2], F32, kind="ExternalInput")
    d_b3 = nc.dram_tensor("b3", [NC, 1], F32, kind="ExternalInput")
    d_gf = nc.dram_tensor("gf", [G, NGF], F32, kind="ExternalInput")
    d_out = nc.dram_tensor("out", [G, NC], F32, kind="ExternalOutput")

    with tile.TileContext(nc, trace_sim=False) as tc:
        with (
            tc.tile_pool(name="sb", bufs=1) as sb,
            tc.tile_pool(name="dr", bufs=2, space="DRAM") as dr,
        ):
            nc.gpsimd.load_library(LIB_MLP)
            idf = sb.tile([P, P], F32)
            make_identity(nc, idf[:])
            idb = sb.tile([P, P], BF)
            make_identity(nc, idb[:])
            eps_sb = sb.tile([P, 1], F32)
            nc.vector.memset(eps_sb[:], EPS)
            # static softmax downscale: exp(e - ln 64). The 1/64 cancels in
            # numerator/denominator; keeps fp16 h*exp products under 65504.
            nl64_sb = sb.tile([P, 1], F32)
            nc.vector.memset(nl64_sb[:], -4.15888308)

            srcg_sb = sb.tile([P, NSUB], I32)
            nc.sync.dma_start(srcg_sb[:], d_SRCG[:])
            ghost_sb = sb.tile([P, NW], F32)
            nc.sync.dma_start(ghost_sb[:], d_gh[:])
            bnp_sb = sb.tile([P, L + 1, 2, 2], F32)
            nc.sync.dma_start(bnp_sb[:], d_bnp[:].rearrange("l p b k -> p l b k"))

            hA = sb.tile([P, 2, NLP], BF)
            hB = sb.tile([P, 2, NLP], BF)
            d_prev = dr.tile([P, 2, NLP], BF, tag="prev", bufs=1)

            # pooling preloads (no deps; issued early so the pooling phase
            # doesn't pay for them)
            sg_sb = sb.tile([P, NW * G], BF)
            nc.sync.dma_start(sg_sb[:], d_Sg[:])
            np_sb = sb.tile([P, NW * P // 16], I16)
            nc.sync.dma_start(np_sb[:], d_np[:])
            NRPL = NG * PADG + NLP
            hf_loc = dr.tile([NRPL, H], BF, tag="hfloc", bufs=1)
            zt0 = sb.tile([P, 2048], BF)
            nc.vector.memset(zt0[:], 0.0)
            for r0 in range(0, NRPL - 1024 + 1, 1024):
                nc.sync.dma_start(
                    hf_loc[r0:r0 + 1024, :].rearrange("(a b) h -> a (b h)", a=P),
                    zt0[:])
            if (NRPL // 1024) * 1024 < NRPL:
                nc.sync.dma_start(
                    hf_loc[NRPL - 1024:NRPL, :].rearrange("(a b) h -> a (b h)", a=P),
                    zt0[:])

            with tc.tile_pool(name="zp", bufs=1) as zp:
                zT = zp.tile([P, 2, NLP], F32)

                def batchnorm_relu(lay, dst_tile, scratch):
                    NCK = 4
                    CK = NLP // NCK
                    stats4 = zp.tile([P, NCK, 4], F32, tag="bnstats4", bufs=2)
                    for ck in range(NCK):
                        c0, c1 = ck * CK, (ck + 1) * CK
                        nc.vector.reduce_sum(stats4[:, ck, 0:1], zT[:, 0, c0:c1], axis=AX.X)
                        nc.vector.reduce_sum(stats4[:, ck, 1:2], zT[:, 1, c0:c1], axis=AX.X)
                        nc.scalar.activation(scratch[:, 0, c0:c1], zT[:, 0, c0:c1], AF.Square,
                                             accum_out=stats4[:, ck, 2:3])
                        nc.scalar.activation(scratch[:, 1, c0:c1], zT[:, 1, c0:c1], AF.Square,
                                             accum_out=stats4[:, ck, 3:4])
                    stats = zp.tile([P, 4], F32, tag="bnstats", bufs=2)
                    nc.vector.tensor_add(stats[:], stats4[:, 0, :], stats4[:, 1, :])
                    nc.vector.tensor_add(stats4[:, 2, :], stats4[:, 2, :], stats4[:, 3, :])
                    nc.vector.tensor_add(stats[:], stats[:], stats4[:, 2, :])
                    sin = dr.tile([P, 4], F32, tag="bnin")
                    sout = dr.tile([P, 4], F32, tag="bnout", addr_space="Shared")
                    nc.sync.dma_start(sin[:], stats[:])
                    nc.gpsimd.collective_compute("AllReduce", ALU.add, replica_groups=RG,
                                                 ins=[sin[:].opt()], outs=[sout[:].opt()])
                    st = zp.tile([P, 4], F32, tag="bnst", bufs=2)
                    nc.sync.dma_start(st[:], sout[:])
                    mu = zp.tile([P, 2], F32, tag="bnmu", bufs=2)
                    nc.vector.tensor_scalar_mul(mu[:], st[:, 0:2], 1.0 / N)
                    var = zp.tile([P, 2], F32, tag="bnvar", bufs=2)
                    nc.vector.tensor_scalar_mul(var[:], st[:, 2:4], 1.0 / N)
                    musq = zp.tile([P, 2], F32, tag="bnmusq", bufs=2)
                    nc.vector.tensor_mul(musq[:], mu[:], mu[:])
                    nc.vector.tensor_sub(var[:], var[:], musq[:])
                    rs = zp.tile([P, 2], F32, tag="bnrs", bufs=2)
                    nc.scalar.activation(rs[:], var[:], AF.Sqrt, bias=eps_sb[:, 0:1])
                    nc.vector.reciprocal(rs[:], rs[:])
                    Sc = zp.tile([P, 2], F32, tag="bnS", bufs=2)
                    nc.vector.tensor_mul(Sc[:], rs[:], bnp_sb[:, lay, :, 0])
                    Bi = zp.tile([P, 2], F32, tag="bnB", bufs=2)
                    nc.vector.tensor_mul(Bi[:], mu[:], Sc[:])
                    nc.vector.tensor_sub(Bi[:], bnp_sb[:, lay, :, 1], Bi[:])
                    for cc in range(4):
                        c0, c1 = cc * (NLP // 4), (cc + 1) * (NLP // 4)
                        for b in range(2):
                            nc.scalar.activation(dst_tile[:, b, c0:c1], zT[:, b, c0:c1], AF.Relu,
                                                 bias=Bi[:, b:b + 1], scale=Sc[:, b:b + 1])

                # ---------------- encoder ----------------
                with (
                    tc.tile_pool(name="encp", bufs=1) as ep,
                    tc.tile_pool(name="psenc", bufs=2, space="PSUM") as ps_enc,
                ):
                    xT_sb = ep.tile([F_IN, NLP], BF)
                    nc.sync.dma_start(xT_sb[:], d_xT[:])
                    wenc_sb = ep.tile([F_IN, H], BF)
                    nc.sync.dma_start(wenc_sb[:], d_Wenc[:])
                    for ch in range(NCH):
                        f0 = ch * 512
                        F = min(512, NLP - f0)
                        for kb in range(2):
                            pz = ps_enc.tile([P, 512], F32, tag="mm")
                            nc.tensor.matmul(pz[:, :F], wenc_sb[:, kb * P:(kb + 1) * P],
                                             xT_sb[:, f0:f0 + F], start=True, stop=True)
                            nc.scalar.activation(zT[:, kb, f0:f0 + F], pz[:, :F], AF.Copy)
                    batchnorm_relu(0, hA, hB)

                # ---------------- GAT layers ----------------
                with (
                    tc.tile_pool(name="edge", bufs=1) as eb,
                    tc.tile_pool(name="pspj", bufs=2, space="PSUM") as ps_pj,
                    tc.tile_pool(name="pswin", bufs=2, space="PSUM") as ps_win,
                    tc.tile_pool(name="pstr", bufs=2, space="PSUM") as ps_tr,
                ):
                    adst = eb.tile([P, NW, HEADS], BF, tag="adst", bufs=2)
                    for l in range(L):
                        hin = hA if l % 2 == 0 else hB
                        hout = hB if l % 2 == 0 else hA
                        wg_sb = eb.tile([P, 2, DHX], BF, tag="wg", bufs=2)
                        nc.sync.dma_start(wg_sb[:], d_WgWa[l].rearrange("t p k -> p t k"))

                        # fused projection: node-major [128, 272] per window
                        hx_loc = dr.tile([NLP, DROW], BF, tag="hxloc")
                        for w in range(NW):
                            n0 = w * WIN
                            pz = ps_pj.tile([P, DHX], F32, tag="pj")
                            for jt in range(2):
                                nc.tensor.matmul(pz[:], hin[:, jt, n0:n0 + P],
                                                 wg_sb[:, jt, :], start=(jt == 0), stop=(jt == 1))
                            hxw = eb.tile([P, DHX], BF, tag="hxw", bufs=3)
                            nc.scalar.activation(hxw[:], pz[:], AF.Copy)
                            nc.vector.tensor_copy(adst[:, w, :], hxw[:, H + HEADS:DHX])
                            nc.sync.dma_start(hx_loc[n0:n0 + P, :], hxw[:, 0:DROW])

                        hx_full = dr.tile([NCORES, NLP, DROW], BF, tag="hxfull", addr_space="Shared")
                        nc.gpsimd.collective_compute("AllGather", ALU.bypass, replica_groups=RG,
                                                     ins=[hx_loc[:].opt()],
                                                     outs=[hx_full[:].opt()])
                        tab = hx_full[:].rearrange("c n d -> (c n) d")

                        for w in range(NW):
                            sst = eb.tile([P, 2, SPW * P], F8, tag="sst", bufs=6)
                            nc.sync.dma_start(sst[:].rearrange("p a b -> p (a b)"), d_SST[:, w])
                            Gt = eb.tile([P, SPW, DROW], BF, tag="G", bufs=6)
                            # subtile 0 = self-loops: the window's own local
                            # rows, one contiguous HWDGE DMA (no AG dep, no Q7)
                            nc.sync.dma_start(Gt[:, 0, :], hx_loc[w * WIN:(w + 1) * WIN, :])
                            for k in range(1, SPW):
                                nc.gpsimd.indirect_dma_start(
                                    out=Gt[:, k, :], out_offset=None,
                                    in_=tab,
                                    in_offset=bass.IndirectOffsetOnAxis(
                                        ap=srcg_sb[:, w * SPW + k:w * SPW + k + 1], axis=0))
                            if debug and l == 0 and w == 0:
                                nc.sync.dma_start(d_dbg_G[:], Gt[:].rearrange("p j d -> p (j d)"))
                            T = ps_win.tile([P, 264 + SPW * HEADS], F32, tag="win")
                            for k in range(SPW):
                                nc.tensor.matmul(T[:, 264 + k * HEADS:264 + (k + 1) * HEADS],
                                                 sst[:, 1, k * P:(k + 1) * P],
                                                 adst[:, w, :], start=True, stop=True)
                            asc = eb.tile([P, SPW, HEADS], F32, tag="asc", bufs=3)
                            nc.vector.tensor_copy(asc[:], Gt[:, :, H:H + HEADS])
                            et = eb.tile([P, SPW, HEADS], F32, tag="et", bufs=3)
                            nc.vector.tensor_add(
                                et[:], asc[:],
                                T[:, 264:264 + SPW * HEADS].rearrange("p (j h) -> p j h", h=HEADS))
                            et3 = eb.tile([P, SPW, HEADS], F32, tag="et3", bufs=3)
                            nc.vector.tensor_scalar_mul(et3[:], et[:], 0.2)
                            nc.vector.tensor_max(et3[:], et3[:], et[:])
                            etb = eb.tile([P, SPW, HEADS], BF, tag="etb", bufs=2)
                            nc.scalar.activation(etb[:], et3[:], AF.Exp, bias=nl64_sb[:, 0:1])
                            # expanded exp on the scalar engine keeps the big
                            # DVE multiply fully contiguous
                            exb = eb.tile([P, SPW, HEADS, HD], BF, tag="exb", bufs=2)
                            nc.scalar.activation(
                                exb[:], et3[:, :, :, None].to_broadcast([P, SPW, HEADS, HD]),
                                AF.Exp, bias=nl64_sb[:, 0:1])
                            # exp into the a_src slot: feeds the denominator
                            # columns of the scatter-add matmul
                            nc.vector.tensor_copy(Gt[:, :, H:H + HEADS], etb[:])
                            gv = Gt[:, :, 0:H].rearrange("p j (h d) -> p j h d", h=HEADS)
                            nc.vector.tensor_mul(gv[:], gv[:], exb[:])
                            for k in range(SPW):
                                nc.tensor.matmul(T[:, 0:264], sst[:, 0, k * P:(k + 1) * P],
                                                 Gt[:, k, 0:264],
                                                 start=(k == 0), stop=(k == SPW - 1))
                            den = eb.tile([P, HEADS], F32, tag="den", bufs=3)
                            nc.vector.tensor_scalar(den[:], T[:, 256:264],
                                                    scalar1=ghost_sb[:, w:w + 1],
                                                    scalar2=None, op0=ALU.add)
                            nc.vector.reciprocal(den[:], den[:])
                            rcx = eb.tile([P, HEADS, HD], F32, tag="rcx", bufs=3)
                            nc.scalar.activation(
                                rcx[:], den[:, :, None].to_broadcast([P, HEADS, HD]), AF.Copy)
                            zw = eb.tile([P, H], F32, tag="zw", bufs=3)
                            nc.vector.tensor_mul(
                                zw[:].rearrange("p (h d) -> p h d", h=HEADS),
                                T[:, 0:H].rearrange("p (h d) -> p h d", h=HEADS),
                                rcx[:])
                            for b in range(2):
                                pt = ps_tr.tile([P, P], F32, tag="trf")
                                nc.tensor.transpose(out=pt[:], in_=zw[:, b * P:(b + 1) * P],
                                                    identity=idf[:])
                                if b == 0:
                                    nc.scalar.activation(zT[:, b, w * WIN:w * WIN + P], pt[:], AF.Copy)
                                else:
                                    nc.vector.tensor_copy(zT[:, b, w * WIN:w * WIN + P], pt[:])

                        if debug and l == 0:
                            nc.sync.dma_start(d_dbg_hx[:], hx_loc[:])
                            nc.sync.dma_start(d_dbg_ad[:], adst[:].rearrange("p w h -> p (w h)"))
                            nc.sync.dma_start(d_dbg_zT[:], zT[:].rearrange("p b n -> p (b n)"))
                        batchnorm_relu(l + 1, hout, hin)
                        if l == 1:
                            nc.sync.dma_start(d_prev[:], hout[:])
                        if l == 2:
                            for b in range(2):
                                for cc in range(7):
                                    c0 = cc * 896
                                    cw = min(896, NLP - c0)
                                    psc = eb.tile([P, 896], BF, tag="prevc", bufs=4)
                                    nc.sync.dma_start(psc[:, :cw], d_prev[:, b, c0:c0 + cw])
                                    nc.vector.tensor_add(hout[:, b, c0:c0 + cw], hout[:, b, c0:c0 + cw],
                                                         psc[:, :cw])

            # ---------------- pooling ----------------
            hfin = hA if L % 2 == 0 else hB
            with (
                tc.tile_pool(name="poolp", bufs=1) as pb,
                tc.tile_pool(name="psmm", bufs=2, space="PSUM") as ps_mm,
                tc.tile_pool(name="pstr2", bufs=2, space="PSUM") as ps_tr,
            ):
                hstage = pb.tile([P, NW, H], BF)
                pp0 = ps_mm.tile([G, H], F32, tag="mm")
                for w in range(NW):
                    n0 = w * WIN
                    for b in range(2):
                        pt = ps_tr.tile([P, P], BF, tag="trb", bufs=2)
                        nc.tensor.transpose(out=pt[:], in_=hfin[:, b, n0:n0 + P], identity=idb[:])
                        nc.vector.tensor_copy(hstage[:, w, b * P:(b + 1) * P], pt[:])
                    nc.tensor.matmul(pp0[:], sg_sb[:, w * G:(w + 1) * G], hstage[:, w, :],
                                     start=(w == 0), stop=(w == NW - 1))

                # per-graph max: scatter-add node rows into the zeroed
                # graph-slot-padded buffer (h >= 0 post-ReLU, so 0-pads are
                # neutral for max), then dma-transpose each slot and reduce
                HWN = 24 * P
                nc.gpsimd.dma_scatter_add(
                    hf_loc[:], hstage[:, 0:24, :], np_sb[:, 0:HWN // 16],
                    HWN, HWN, H)
                nc.gpsimd.dma_scatter_add(
                    hf_loc[:], hstage[:, 24:NW, :], np_sb[:, HWN // 16:NW * P // 16],
                    (NW - 24) * P, (NW - 24) * P, H)

                pmax = pb.tile([P, 2, NG], F32)
                for i in range(NG):
                    gt = pb.tile([P, 2, PADG], BF, tag="gt", bufs=2)
                    for b in range(2):
                        nc.sync.dma_start_transpose(
                            gt[:, b, :], hf_loc[i * PADG:(i + 1) * PADG, b * P:(b + 1) * P])
                    for b in range(2):
                        nc.vector.reduce_max(pmax[:, b, i:i + 1], gt[:, b, :], axis=AX.X)

                # rows: [G, 2H] = [sum | max]; combine across cores with one AG
                pmax_rows = pb.tile([P, H], F32)
                for b in range(2):
                    pt = ps_tr.tile([P, P], F32, tag="trf")
                    nc.tensor.transpose(out=pt[0:NG, 0:P], in_=pmax[:, b, :], identity=idf[:])
                    nc.scalar.activation(pmax_rows[0:NG, b * P:(b + 1) * P], pt[0:NG, 0:P], AF.Copy)

                pin_sb = pb.tile([G, 2 * H], F32)
                nc.scalar.activation(pin_sb[:, 0:H], pp0[:], AF.Copy)
                nc.vector.memset(pin_sb[:, H:2 * H], 0.0)
                pin = dr.tile([G + 1, 2 * H], F32, tag="pin")
                nc.sync.dma_start(pin[0:G, :], pin_sb[:])
                gtgt_sb = pb.tile([NG, 1], I32)
                nc.sync.dma_start(gtgt_sb[:], d_gtgt[:])
                nc.gpsimd.indirect_dma_start(
                    out=pin[:], out_offset=bass.IndirectOffsetOnAxis(ap=gtgt_sb[:], axis=0),
                    in_=pmax_rows[0:NG, :], in_offset=None, element_offset=H)

                pfull_d = dr.tile([NCORES, G, 2 * H], F32, tag="pfull", addr_space="Shared")
                nc.gpsimd.collective_compute("AllGather", ALU.bypass, replica_groups=RG,
                                             ins=[pin[0:G, :].opt()], outs=[pfull_d[:].opt()])
                pf8 = pb.tile([G, NCORES, 2 * H], F32)
                nc.sync.dma_start(pf8[:], pfull_d[:].rearrange("c g h -> g c h"))
                for cc in range(1, NCORES):
                    nc.vector.tensor_add(pf8[:, 0, 0:H], pf8[:, 0, 0:H], pf8[:, cc, 0:H])
                    nc.vector.tensor_max(pf8[:, 0, H:2 * H], pf8[:, 0, H:2 * H], pf8[:, cc, H:2 * H])

                # pooled [G, 800] = [mean | max | sum | gf]
                pooled = pb.tile([G, 3 * H + NGF], F32)
                cnt_sb = pb.tile([G, 1], F32)
                nc.sync.dma_start(cnt_sb[:], d_cnt[:])
                nc.vector.tensor_scalar(pooled[:, 0:H], pf8[:, 0, 0:H], scalar1=cnt_sb[:],
                                        scalar2=None, op0=ALU.mult)
                nc.vector.tensor_copy(pooled[:, H:2 * H], pf8[:, 0, H:2 * H])
                nc.vector.tensor_copy(pooled[:, 2 * H:3 * H], pf8[:, 0, 0:H])
                nc.sync.dma_start(pooled[:, 3 * H:], d_gf[:])

                # ---------------- classifier (replicated) ----------------
                pT = pb.tile([P, 7, G], F32)
                nc.vector.memset(pT[:], 0)
                for t in range(7):
                    w_ = min(P, 3 * H + NGF - t * P)
                    pt = ps_mm.tile([P, P], F32, tag="mm")
                    nc.tensor.transpose(out=pt[0:w_, 0:G], in_=pooled[:, t * P:t * P + w_],
                                        identity=idf[0:G, 0:G])
                    nc.scalar.activation(pT[:w_, t, :], pt[:w_, 0:G], AF.Copy)

                w1_sb = pb.tile([P, 7, 2 * H], F32)
                nc.sync.dma_start(w1_sb[:], d_W1[:].rearrange("t p k -> p t k"))
                bn1_sb = pb.tile([P, 4, 2], F32)
                nc.sync.dma_start(bn1_sb[:], d_bn1[:])
                z1 = pb.tile([P, 4, G], F32)

                def mlp_bn(zt, nblk, bnsb, ngraph=G):
                    for b in range(nblk):
                        s_ = pb.tile([P, 1], F32, tag="cbs", bufs=2)
                        nc.vector.reduce_sum(s_[:], zt[:, b, :], axis=AX.X)
                        sqt = pb.tile([P, G], F32, tag="cbsq", bufs=2)
                        q_ = pb.tile([P, 1], F32, tag="cbq", bufs=2)
                        nc.scalar.activation(sqt[:], zt[:, b, :], AF.Square, accum_out=q_[:])
                        mu = pb.tile([P, 1], F32, tag="cbmu", bufs=2)
                        nc.vector.tensor_scalar_mul(mu[:], s_[:], 1.0 / ngraph)
                        var = pb.tile([P, 1], F32, tag="cbvar", bufs=2)
                        nc.vector.tensor_scalar_mul(var[:], q_[:], 1.0 / ngraph)
                        ms = pb.tile([P, 1], F32, tag="cbms", bufs=2)
                        nc.vector.tensor_mul(ms[:], mu[:], mu[:])
                        nc.vector.tensor_sub(var[:], var[:], ms[:])
                        rs = pb.tile([P, 1], F32, tag="cbrs", bufs=2)
                        nc.scalar.activation(rs[:], var[:], AF.Sqrt, bias=eps_sb[:, 0:1])
                        nc.vector.reciprocal(rs[:], rs[:])
                        Sc = pb.tile([P, 1], F32, tag="cbS", bufs=2)
                        nc.vector.tensor_mul(Sc[:], rs[:], bnsb[:, b, 0:1])
                        Bi = pb.tile([P, 1], F32, tag="cbB", bufs=2)
                        nc.vector.tensor_mul(Bi[:], mu[:], Sc[:])
                        nc.vector.tensor_sub(Bi[:], bnsb[:, b, 1:2], Bi[:])
                        nc.scalar.activation(zt[:, b, :], zt[:, b, :], AF.Relu,
                                             bias=Bi[:], scale=Sc[:])

                for mb in range(4):
                    pz = ps_mm.tile([P, 512], F32, tag="mm")
                    for kt in range(7):
                        nc.tensor.matmul(pz[:, 0:G], w1_sb[:, kt, mb * P:(mb + 1) * P],
                                         pT[:, kt, :], start=(kt == 0), stop=(kt == 6))
                    nc.scalar.activation(z1[:, mb, :], pz[:, 0:G], AF.Copy)
                mlp_bn(z1, 4, bn1_sb)

                w2_sb = pb.tile([P, 4, H], F32)
                nc.sync.dma_start(w2_sb[:], d_W2[:].rearrange("t p k -> p t k"))
                bn2_sb = pb.tile([P, 2, 2], F32)
                nc.sync.dma_start(bn2_sb[:], d_bn2[:])
                z2 = pb.tile([P, 2, G], F32)
                for mb in range(2):
                    pz = ps_mm.tile([P, 512], F32, tag="mm")
                    for kt in range(4):
                        nc.tensor.matmul(pz[:, 0:G], w2_sb[:, kt, mb * P:(mb + 1) * P],
                                         z1[:, kt, :], start=(kt == 0), stop=(kt == 3))
                    nc.scalar.activation(z2[:, mb, :], pz[:, 0:G], AF.Copy)
                mlp_bn(z2, 2, bn2_sb)

                w3_sb = pb.tile([P, 2, NC], F32)
                nc.sync.dma_start(w3_sb[:], d_W3[:].rearrange("t p k -> p t k"))
                b3_sb = pb.tile([NC, 1], F32)
                nc.sync.dma_start(b3_sb[:], d_b3[:])
                pz3 = ps_mm.tile([P, 512], F32, tag="mm")
                for kt in range(2):
                    nc.tensor.matmul(pz3[0:NC, 0:G], w3_sb[:, kt, :], z2[:, kt, :],
                                     start=(kt == 0), stop=(kt == 1))
                z3 = pb.tile([NC, G], F32)
                nc.scalar.activation(z3[:], pz3[0:NC, 0:G], AF.Identity, bias=b3_sb[:, 0:1])
                nc.sync.dma_start(d_out[:].rearrange("g c -> c g"), z3[:])
    return nc


_CACHE = {}


def _get_compiled(cfg, debug=False):
    key = (cfg["SPW"], cfg["NSUB"], cfg["NG"], debug)
    if key not in _CACHE:
        nc = bacc.Bacc("TRN2", target_bir_lowering=False, debug=False,
                       num_devices=NCORES, dynamic_dma_scratch_size=24576)
        _build(nc, cfg, debug=debug)
        nc.compile()
        _CACHE[key] = nc
    return _CACHE[key]


def kernel(debug=False, _want_results=False, **inputs):
    in_maps, cfg = _prep(inputs)
    nc = _get_compiled(cfg)
    res = run_bass_kernel_spmd(nc, in_maps, core_ids=list(range(NCORES)))
    out = np.asarray(res.results[0]["out"], np.float32)
    if _want_results:
        return out, res
    return out
